# revision 3
# baseline (speedup 1.0000x reference)
"""Distributed Trainium2 Bass kernel for nn_ClosedFlyLoop.

Strategy (8 NeuronCores, shard X into 8 blocks of 256):
 - host: symmetrize v, split y into (m[4], s), pad X edge-replicate by H=25,
   cut per-core overlapping slabs [7, 1024, 306] (halo covers smooth+diff+
   post-smooth supports; the AP_CUT zeroing erases the only global-edge
   discrepancy), convert to bf16.
 - device, per core (no collectives needed):
     stage 1a: conv along y (circular) as banded matmuls that also transpose
               layout [y,x] -> [x,y']  (lhsT = data chunk, rhs = Toeplitz band)
     stage 1b: conv along x as banded matmuls transposing back [x,y]->[y,x'],
               producing gradient pairs in PSUM (sign/scale folds baked into
               the band matrices)
     algebra:  pointwise ClosedFlyLoop RHS; the v-and-friends products read
               their gradient operands STRAIGHT FROM PSUM (no evacuation
               copies), work split across DVE / Pool-STT / ACT by measured
               cost-model rates
     stage 4a/4b: final Gaussian smooth of the 5 masked pre-fields, 2-bank
               [128,1024] psum groups with single-instruction evacs, mask
               folded into the 4a evac as a per-partition scalar multiply,
               4b outputs packed 2-tiles-per-bank
 - host: concatenate per-core [5, 1024, 256] bf16 outputs along X, upcast.
"""
import numpy as np
import ml_dtypes

import concourse.bass as bass
import concourse.bacc as bacc
import concourse.mybir as mybir
from concourse import tile
from concourse.bass_utils import run_bass_kernel_spmd

BF16 = ml_dtypes.bfloat16
F32 = np.float32

Y, X = 1024, 2048
NCORES = 8
XS = X // NCORES            # 256
RAD = 12                    # gauss radius: int(4.0*3.0+0.5)
H = 2 * RAD + 1             # 25
W_IN = XS + 2 * H           # 306
W_ALG = XS + 2 * RAD        # 280
OFF1B = H - RAD             # 13: slab-coord offset of alg window
AP_CUT = 15
YT = Y // 128               # 8 y tiles
XT_IN = [(0, 128), (128, 128), (256, W_IN - 256)]       # x tiles of slab (128,128,50)
XT_ALG = [(0, 128), (128, 128), (256, W_ALG - 256)]     # x tiles of alg width (128,128,24)
HALF = 512                  # y' half width for stage a psum


def _gauss():
    r = RAD
    x = np.arange(-r, r + 1, dtype=np.float64)
    k = np.exp(-0.5 * (x / 3.0) ** 2)
    k = (k / k.sum()).astype(np.float64)
    dk = np.convolve(k, [-0.5, 0.0, 0.5])
    return k.astype(F32), dk.astype(F32)


KERN, DKERN = _gauss()      # 25 taps (rad 12), 27 taps (rad 13)


# ---------------- band submatrix machinery (host) ----------------
class BandPack:
    """Dedup + pack all band submatrices into one [128, K] bf16 constant."""

    def __init__(self):
        self.blocks = {}
        self.cols = []
        self.total = 0

    def add(self, sub):
        sub16 = np.ascontiguousarray(sub.astype(BF16))
        key = (sub16.shape, sub16.tobytes())
        if key not in self.blocks:
            pad = np.zeros((128, sub16.shape[1]), dtype=BF16)
            pad[: sub16.shape[0]] = sub16
            self.blocks[key] = self.total
            self.cols.append(pad)
            self.total += sub16.shape[1]
        return self.blocks[key]

    def packed(self):
        return np.concatenate(self.cols, axis=1)


def band_subs_y(pack, ker, scale):
    """Circular conv along y (1024). Returns per half h: list of
    (chunk k, col a, col b, packed offset, rows) for rhs = B[krows, h*512+a : h*512+b]."""
    r = ker.shape[0] // 2
    B = np.zeros((Y, Y), dtype=F32)
    for j in range(Y):
        for t in range(-r, r + 1):
            B[(j + t) % Y, j] = ker[r + t] * scale
    out = []
    for h in range(2):
        subs = []
        for k in range(YT):
            sub = B[k * 128:(k + 1) * 128, h * HALF:(h + 1) * HALF]
            cols = np.flatnonzero(np.any(sub != 0.0, axis=0))
            if cols.size == 0:
                continue
            a, b = int(cols[0]), int(cols[-1] + 1)
            assert b - a == cols.size
            off = pack.add(sub[:, a:b])
            subs.append((k, a, b, off, 128))
        out.append(subs)
    return out


def band_subs_x(pack, ker, scale, n_in, n_out, off_in):
    """conv along x: out[j] = sum_t kc[t] in[j + off_in + t].
    Returns list of (chunk k, col a, col b, packed offset, rows)."""
    r = ker.shape[0] // 2
    B = np.zeros((n_in, n_out), dtype=F32)
    for j in range(n_out):
        for t in range(-r, r + 1):
            i = j + off_in + t
            if 0 <= i < n_in:
                B[i, j] = ker[r + t] * scale
    subs = []
    nchunks = (n_in + 127) // 128
    for k in range(nchunks):
        rows = min(128, n_in - k * 128)
        sub = B[k * 128:k * 128 + rows, :]
        cols = np.flatnonzero(np.any(sub != 0.0, axis=0))
        if cols.size == 0:
            continue
        a, b = int(cols[0]), int(cols[-1] + 1)
        assert b - a == cols.size
        off = pack.add(sub[:, a:b])
        subs.append((k, a, b, off, rows))
    return subs


# channel order in slab: m00 m01 m10 m11 s v0 v1
# stage-1a D-variant scale per channel (folds signs), stage-1b DK scale per channel
CH_DY_SCALE = [-1.0, -1.0, -1.0, -1.0, -1.0, 1.0, 0.5]   # conv_y(DK) scale
CH_DX_SCALE = [-1.0, -1.0, -1.0, -1.0, -1.0, -0.5, 1.0]  # conv_x(DK) scale


def build_graph():
    pack = BandPack()
    sub_ky = band_subs_y(pack, KERN, 1.0)                 # shared smooth-y (also stage 4a)
    sub_kx = band_subs_x(pack, KERN, 1.0, W_IN, W_ALG, OFF1B)    # smooth-x for dy fields
    # dy via raw 3-tap diff of gyk: fold 0.5*CH_DY_SCALE into the 1b Kx band
    sub_kxd = {}
    for sc in sorted(set(0.5 * s for s in CH_DY_SCALE)):
        sub_kxd[sc] = band_subs_x(pack, KERN, sc, W_IN, W_ALG, OFF1B)
    sub_dkx = {}
    for sc in sorted(set(CH_DX_SCALE)):
        sub_dkx[sc] = band_subs_x(pack, DKERN, sc, W_IN, W_ALG, OFF1B)
    sub_kx4 = band_subs_x(pack, KERN, 1.0, W_ALG, XS, RAD)       # stage 4b
    bands_np = pack.packed()
    KTOT = bands_np.shape[1]

    nc = bacc.Bacc()
    x_ext = nc.declare_dram_parameter("x", [7, Y, W_IN], mybir.dt.bfloat16, isOutput=False)
    bands_ext = nc.declare_dram_parameter("bands", [128, KTOT], mybir.dt.bfloat16, isOutput=False)
    mask_ext = nc.declare_dram_parameter("mask", [128, 3], mybir.dt.float32, isOutput=False)
    out_ext = nc.declare_dram_parameter("out", [5, Y, XS], mybir.dt.bfloat16, isOutput=True)

    bf = mybir.dt.bfloat16
    f32 = mybir.dt.float32
    TT = mybir.AluOpType
    ACT_COPY = mybir.ActivationFunctionType.Copy

    with tile.TileContext(nc) as tc:
        with (
            tc.tile_pool(name="const", bufs=1) as constp,
            tc.tile_pool(name="slab", bufs=1) as slabp,
            tc.tile_pool(name="gyt", bufs=1) as gytp,
            tc.tile_pool(name="alg", bufs=2) as algp,
            tc.tile_pool(name="pre", bufs=1) as prep,
            tc.tile_pool(name="gyt2", bufs=3) as gyt2p,
            tc.tile_pool(name="outs", bufs=2) as outsp,
            tc.tile_pool(name="psa", bufs=2, space=bass.MemorySpace.PSUM) as psap,
            tc.tile_pool(name="psg", bufs=2, space=bass.MemorySpace.PSUM) as psgp,
        ):
            bands = constp.tile([128, KTOT], bf, tag="bands", name="bands")
            nc.sync.dma_start(bands[:, :], bands_ext[:, :])
            maskt = constp.tile([128, 3], f32, tag="mask", name="mask")
            nc.sync.dma_start(maskt[:, :], mask_ext[:, :])

            # persistent slab: one wide tile per channel [128, YT*W_IN]; one DMA
            # per channel (128 descriptors of 8*612B rows vs 8 DMAs of 625ns
            # HWDGE overhead each).
            slabw = [slabp.tile([128, YT * W_IN], bf, tag=f"slabw{c}", name=f"slabw{c}")
                     for c in range(7)]
            for c in range(7):
                nc.sync.dma_start(
                    slabw[c][:, :],
                    x_ext[c].rearrange("(t p) x -> p t x", p=128))
            slab = [[slabw[c][:, t * W_IN:(t + 1) * W_IN] for t in range(YT)]
                    for c in range(7)]

            def conv_group(psum_ap, subs, lhsT_fn):
                n = len(subs)
                for i, (k, a, b, off, rows) in enumerate(subs):
                    nc.tensor.matmul(
                        psum_ap[:, a:b],
                        lhsT_fn(k, rows),
                        bands[:rows, off:off + b - a],
                        start=(i == 0),
                        stop=(i == n - 1),
                    )

            # ---------------- stage 1a: conv_y (Ky only) for all channels ----------------
            # dy rides a raw circular 3-tap diff of gyk along the free y dim;
            # the 0.5 and per-channel signs are folded into the 1b Kx bands.
            # 2-bank psum tiles: both halves land in one [128,1024] with a
            # single-instruction evac on ACT.
            gyk_all, gyd_all = [], []
            for c in range(7):
                gyk = [gytp.tile([128, 1024], bf, tag=f"gyk{c}_{xt}", name=f"gyk{c}_{xt}") for xt in range(3)]
                gyd = [gytp.tile([128, 1024], bf, tag=f"gyd{c}_{xt}", name=f"gyd{c}_{xt}") for xt in range(3)]
                gyk_all.append(gyk); gyd_all.append(gyd)
                for xt, (x0, xw) in enumerate(XT_IN):
                    ps = psap.tile([128, 2 * HALF], f32, tag="psa", name="psa")
                    for h in range(2):
                        conv_group(
                            ps[:xw, h * HALF:(h + 1) * HALF], sub_ky[h],
                            lambda k, rows: slab[c][k][:, x0:x0 + xw])
                    nc.scalar.copy(gyk[xt][:xw, :], ps[:xw, :])
                    g, d = gyk[xt], gyd[xt]
                    nc.vector.tensor_tensor(d[:xw, 1:1023], g[:xw, 2:1024], g[:xw, 0:1022], TT.subtract)
                    nc.gpsimd.tensor_tensor(d[:xw, 0:1], g[:xw, 1:2], g[:xw, 1023:1024], TT.subtract)
                    nc.gpsimd.tensor_tensor(d[:xw, 1023:1024], g[:xw, 0:1], g[:xw, 1022:1023], TT.subtract)

            # ------- stage 1b + algebra, y-tile major: gradients consumed from PSUM -------
            pre = [[prep.tile([128, W_ALG], bf, tag=f"pre{f}_{t}", name=f"pre{f}_{t}") for t in range(YT)]
                   for f in range(5)]
            A = slice(OFF1B, OFF1B + W_ALG)
            dxsubs = {c: sub_dkx[CH_DX_SCALE[c]] for c in range(7)}

            def _emit(ps_view, t, specs):
                first = True
                for gi, (gt, subs) in enumerate(specs):
                    n = len(subs)
                    for i, (k, a, b, off, rows) in enumerate(subs):
                        nc.tensor.matmul(
                            ps_view[:, a:b],
                            gt[k][:rows, t * 128:(t + 1) * 128],
                            bands[:rows, off:off + b - a],
                            start=first,
                            stop=(gi == len(specs) - 1 and i == n - 1),
                        )
                        first = False

            def grad_pair(t, specsA, specsB):
                """Two conv groups into one 2-bank psum tile (separate zero
                regions at f32 cols 0 and 512); consumed in place."""
                ps = psgp.tile([128, 2 * HALF], f32, tag="pg", name="pg")
                _emit(ps[:, 0:HALF], t, specsA)
                _emit(ps[:, HALF:2 * HALF], t, specsB)
                return ps

            # Pool fused op: out = (in0 * 1) op in1 -- InstTensorScalarPtr runs
            # at the 0.60 default gpsimd efficiency vs 0.42 for TensorTensor.
            def pool_tt(out_ap, in0_ap, in1_ap, op):
                nc.gpsimd.scalar_tensor_tensor(out_ap, in0_ap, 1.0, in1_ap, TT.mult, op)

            for t in range(YT):
                m0a, m1a = slab[0][t][:, A], slab[1][t][:, A]
                m2a, m3a = slab[2][t][:, A], slab[3][t][:, A]
                sa, v0a, v1a = slab[4][t][:, A], slab[5][t][:, A], slab[6][t][:, A]

                def tmp(tag):
                    return algp.tile([128, W_ALG], bf, tag=tag, name=tag)

                # ACT affines (no psum deps)
                c1 = tmp("c1"); c2 = tmp("c2"); c3 = tmp("c3"); c4 = tmp("c4")
                nc.scalar.activation(c1[:, :], sa, ACT_COPY, bias=-0.11, scale=0.099)
                nc.scalar.activation(c2[:, :], sa, ACT_COPY, bias=0.767, scale=0.055)
                nc.scalar.activation(c3[:, :], sa, ACT_COPY, bias=0.732, scale=-0.59)
                nc.scalar.activation(c4[:, :], sa, ACT_COPY, bias=0.069, scale=-0.048)
                # Pool: raw combinations
                trm = tmp("trm"); u1 = tmp("u1"); u2 = tmp("u2")
                pool_tt(trm[:, :], m0a, m3a, TT.add)
                pool_tt(u1[:, :], m1a, m2a, TT.add)
                pool_tt(u2[:, :], m3a, m0a, TT.subtract)

                # w and trE (each a 2-conv-group merge) share one 2-bank psum;
                # both consumed straight from PSUM (w twice, trE once).
                ps_wt = grad_pair(
                    t,
                    [(gyk_all[5], dxsubs[5]), (gyd_all[6], sub_kxd[0.5 * CH_DY_SCALE[6]])],
                    [(gyd_all[5], sub_kxd[0.5 * CH_DY_SCALE[5]]), (gyk_all[6], dxsubs[6])])
                wu1 = tmp("wu1"); wu2 = tmp("wu2")
                nc.vector.tensor_tensor(wu1[:, :], ps_wt[:, 0:W_ALG], u1[:, :], TT.mult)
                nc.gpsimd.scalar_tensor_tensor(wu2[:, :], ps_wt[:, 0:W_ALG], 1.0, u2[:, :], TT.mult, TT.mult)
                t1 = tmp("t1"); t2 = tmp("t2"); Ac = tmp("Ac"); Cc = tmp("Cc")
                nc.vector.tensor_tensor(t1[:, :], ps_wt[:, HALF:HALF + W_ALG], c2[:, :], TT.mult)
                pool_tt(t2[:, :], c3[:, :], trm[:, :], TT.mult)
                nc.vector.tensor_tensor(t1[:, :], t1[:, :], c1[:, :], TT.add)
                nc.vector.tensor_tensor(Ac[:, :], t1[:, :], t2[:, :], TT.add)
                pool_tt(Cc[:, :], c4[:, :], trm[:, :], TT.mult)

                # sdot (folds: psum = -dys, -dxs), products straight from PSUM
                ps_s = grad_pair(t, [(gyd_all[4], sub_kxd[0.5 * CH_DY_SCALE[4]])],
                                 [(gyk_all[4], dxsubs[4])])
                sd1 = tmp("sd1"); sd2 = tmp("sd2")
                nc.vector.tensor_tensor(sd1[:, :], ps_s[:, 0:W_ALG], v0a, TT.mult)
                nc.gpsimd.scalar_tensor_tensor(sd2[:, :], ps_s[:, HALF:HALF + W_ALG], 1.0, v1a, TT.mult, TT.mult)
                nc.vector.tensor_tensor(pre[4][t][:, :], sd1[:, :], sd2[:, :], TT.add)

                mas = (m0a, m1a, m2a, m3a)
                for ch in range(4):
                    ps_ab = grad_pair(t, [(gyd_all[ch], sub_kxd[0.5 * CH_DY_SCALE[ch]])],
                                      [(gyk_all[ch], dxsubs[ch])])
                    q1 = tmp("q1"); q2 = tmp("q2"); r = tmp("r")
                    nc.vector.tensor_tensor(q1[:, :], ps_ab[:, 0:W_ALG], v0a, TT.mult)
                    nc.gpsimd.scalar_tensor_tensor(q2[:, :], ps_ab[:, HALF:HALF + W_ALG], 1.0, v1a, TT.mult, TT.mult)
                    if ch % 2 == 0:
                        nc.vector.tensor_tensor(r[:, :], Ac[:, :], mas[ch], TT.mult)
                    else:
                        pool_tt(r[:, :], Ac[:, :], mas[ch], TT.mult)
                    nc.vector.tensor_tensor(q1[:, :], q1[:, :], q2[:, :], TT.add)
                    nc.vector.tensor_tensor(q1[:, :], q1[:, :], r[:, :], TT.add)
                    p = pre[ch][t]
                    if ch == 0:
                        nc.vector.tensor_tensor(q1[:, :], q1[:, :], wu1[:, :], TT.subtract)
                        nc.vector.tensor_tensor(p[:, :], q1[:, :], Cc[:, :], TT.add)
                    elif ch == 3:
                        nc.vector.tensor_tensor(p[:, :], q1[:, :], wu1[:, :], TT.add)
                    else:
                        nc.vector.tensor_tensor(p[:, :], q1[:, :], wu2[:, :], TT.subtract)

            # ---------------- stage 4: final smooth of 5 fields ----------------
            # 4a: 2-bank psum per (f, xt) with single mask-scaled ACT evac.
            # 4b: pack two y-tiles per psum bank, evac pairs on DVE.
            for f in range(5):
                gy2 = [gyt2p.tile([128, 1024], bf, tag=f"gy2{xt}", name=f"gy2{xt}") for xt in range(3)]
                for xt, (x0, xw) in enumerate(XT_ALG):
                    ps = psap.tile([128, 2 * HALF], f32, tag="psa", name="psa")
                    for h in range(2):
                        conv_group(
                            ps[:xw, h * HALF:(h + 1) * HALF], sub_ky[h],
                            lambda k, rows: pre[f][k][:, x0:x0 + xw])
                    nc.scalar.activation(gy2[xt][:xw, :], ps[:xw, :], ACT_COPY,
                                         scale=maskt[:xw, xt:xt + 1])
                ow = outsp.tile([128, YT * XS], bf, tag="ow", name="ow")
                for tp in range(YT // 2):
                    ps = psap.tile([128, 2 * HALF], f32, tag="psa", name="psa")
                    for half in range(2):
                        t = 2 * tp + half
                        conv_group(
                            ps[:, half * HALF:half * HALF + XS], sub_kx4,
                            lambda k, rows: gy2[k][:rows, t * 128:(t + 1) * 128])
                    dst = ow.rearrange("p (t x) -> p t x", x=XS)[:, 2 * tp:2 * tp + 2, :]
                    src = ps.rearrange("p (t x) -> p t x", x=HALF)[:, :, 0:XS]
                    if f % 2 == 0:
                        nc.vector.tensor_copy(dst, src)
                    else:
                        nc.scalar.copy(dst, src)
                nc.sync.dma_start(
                    out_ext[f].rearrange("(t p) x -> p t x", p=128), ow[:, :])

    nc.compile()
    return nc, bands_np


_CACHE = {}


def _get_graph():
    if "nc" not in _CACHE:
        _CACHE["nc"], _CACHE["bands"] = build_graph()
    return _CACHE["nc"], _CACHE["bands"]


def host_prep(y, v):
    m = y[:4]
    s = y[4:5]
    v_lr = v[:, ::-1, :].copy()
    v_lr[0] *= -1.0
    vs = 0.5 * (v + v_lr)
    f = np.concatenate([m, s, vs], axis=0).astype(F32)      # [7, Y, X]
    fp = np.pad(f, ((0, 0), (0, 0), (H, H)), mode='edge')
    slabs, masks = [], []
    for c in range(NCORES):
        x0 = c * XS
        slabs.append(np.ascontiguousarray(fp[:, :, x0:x0 + W_IN]).astype(BF16))
        g = x0 + np.arange(W_ALG) - RAD
        mk = ((g >= AP_CUT) & (g < X - AP_CUT)).astype(F32)
        mk_t = np.zeros((128, 3), dtype=F32)
        for xt, (a, w) in enumerate(XT_ALG):
            mk_t[:w, xt] = mk[a:a + w]
        masks.append(mk_t)
    return slabs, masks


def kernel(y, v):
    y = np.asarray(y, dtype=F32)
    v = np.asarray(v, dtype=F32)
    nc, bands_np = _get_graph()
    slabs, masks = host_prep(y, v)
    in_maps = [
        {"x": slabs[c], "bands": bands_np, "mask": masks[c]}
        for c in range(NCORES)
    ]
    res = run_bass_kernel_spmd(nc, in_maps, core_ids=list(range(NCORES)))
    out = np.concatenate([res.results[c]["out"] for c in range(NCORES)], axis=2)
    return out.astype(F32)


# revision 7
# speedup vs baseline: 1.0364x; 1.0364x over previous
"""Distributed Trainium2 Bass kernel for nn_ClosedFlyLoop.

Strategy (8 NeuronCores, shard X into 8 blocks of 256):
 - host: symmetrize v, split y into (m[4], s), pad X edge-replicate by H=25,
   cut per-core overlapping slabs [7, 1024, 306] (halo covers smooth+diff+
   post-smooth supports; the AP_CUT zeroing erases the only global-edge
   discrepancy), convert to bf16.
 - device, per core (no collectives needed):
     wide precompute: everything that depends only on the raw slab (the four
               affine-in-s coefficient fields, trm/u1/u2, their combinations)
               is computed once for the whole core as [128, 8*280] strided
               ops, overlapped with the input DMA.
     stage 1a: conv along y (circular) as banded matmuls that also transpose
               layout [y,x] -> [x,y']  (lhsT = data chunk, rhs = Toeplitz
               band); 2-bank [128,1024] psum groups, single-instruction
               evacs split ACT/DVE, dy via circular 3-tap diff split DVE/Pool
     stage 1b: conv along x as banded matmuls transposing back [x,y]->[y,x'],
               producing gradient pairs in PSUM. w/trE and s pairs are
               evacuated by ACT; the four m pairs are consumed straight from
               PSUM (q1 on DVE, q2 on Pool-STT). Adds are tree-structured:
               Pool combines b = r +- w*u off the critical chain.
     stage 4a/4b: final Gaussian smooth of the 5 masked pre-fields, 2-bank
               psum groups, mask folded into the 4a evac as a per-partition
               scalar multiply, 4b outputs packed 2 y-tiles per bank; evacs
               round-robin ACT/DVE/Pool.
 - host: concatenate per-core [5, 1024, 256] bf16 outputs along X, upcast.
"""
import numpy as np
import ml_dtypes

import concourse.bass as bass
import concourse.bacc as bacc
import concourse.mybir as mybir
from concourse import tile
from concourse.bass_utils import run_bass_kernel_spmd

BF16 = ml_dtypes.bfloat16
F32 = np.float32

Y, X = 1024, 2048
NCORES = 8
XS = X // NCORES            # 256
RAD = 12                    # gauss radius: int(4.0*3.0+0.5)
H = 2 * RAD + 1             # 25
W_IN = XS + 2 * H           # 306
W_ALG = XS + 2 * RAD        # 280
OFF1B = H - RAD             # 13: slab-coord offset of alg window
AP_CUT = 15
YT = Y // 128               # 8 y tiles
XT_IN = [(0, 128), (128, 128), (256, W_IN - 256)]       # x tiles of slab (128,128,50)
XT_ALG = [(0, 128), (128, 128), (256, W_ALG - 256)]     # x tiles of alg width (128,128,24)
HALF = 512                  # y' half width for stage a psum
WW = YT * W_ALG             # 2240: wide-op width


def _gauss():
    r = RAD
    x = np.arange(-r, r + 1, dtype=np.float64)
    k = np.exp(-0.5 * (x / 3.0) ** 2)
    k = (k / k.sum()).astype(np.float64)
    dk = np.convolve(k, [-0.5, 0.0, 0.5])
    return k.astype(F32), dk.astype(F32)


KERN, DKERN = _gauss()      # 25 taps (rad 12), 27 taps (rad 13)


# ---------------- band submatrix machinery (host) ----------------
class BandPack:
    """Dedup + pack all band submatrices into one [128, K] bf16 constant."""

    def __init__(self):
        self.blocks = {}
        self.cols = []
        self.total = 0

    def add(self, sub):
        sub16 = np.ascontiguousarray(sub.astype(BF16))
        key = (sub16.shape, sub16.tobytes())
        if key not in self.blocks:
            pad = np.zeros((128, sub16.shape[1]), dtype=BF16)
            pad[: sub16.shape[0]] = sub16
            self.blocks[key] = self.total
            self.cols.append(pad)
            self.total += sub16.shape[1]
        return self.blocks[key]

    def packed(self):
        return np.concatenate(self.cols, axis=1)


def band_subs_y(pack, ker, scale):
    """Circular conv along y (1024). Returns per half h: list of
    (chunk k, col a, col b, packed offset, rows) for rhs = B[krows, h*512+a : h*512+b]."""
    r = ker.shape[0] // 2
    B = np.zeros((Y, Y), dtype=F32)
    for j in range(Y):
        for t in range(-r, r + 1):
            B[(j + t) % Y, j] = ker[r + t] * scale
    out = []
    for h in range(2):
        subs = []
        for k in range(YT):
            sub = B[k * 128:(k + 1) * 128, h * HALF:(h + 1) * HALF]
            cols = np.flatnonzero(np.any(sub != 0.0, axis=0))
            if cols.size == 0:
                continue
            a, b = int(cols[0]), int(cols[-1] + 1)
            assert b - a == cols.size
            off = pack.add(sub[:, a:b])
            subs.append((k, a, b, off, 128))
        out.append(subs)
    return out


def band_subs_x(pack, ker, scale, n_in, n_out, off_in):
    """conv along x: out[j] = sum_t kc[t] in[j + off_in + t].
    Returns list of (chunk k, col a, col b, packed offset, rows)."""
    r = ker.shape[0] // 2
    B = np.zeros((n_in, n_out), dtype=F32)
    for j in range(n_out):
        for t in range(-r, r + 1):
            i = j + off_in + t
            if 0 <= i < n_in:
                B[i, j] = ker[r + t] * scale
    subs = []
    nchunks = (n_in + 127) // 128
    for k in range(nchunks):
        rows = min(128, n_in - k * 128)
        sub = B[k * 128:k * 128 + rows, :]
        cols = np.flatnonzero(np.any(sub != 0.0, axis=0))
        if cols.size == 0:
            continue
        a, b = int(cols[0]), int(cols[-1] + 1)
        assert b - a == cols.size
        off = pack.add(sub[:, a:b])
        subs.append((k, a, b, off, rows))
    return subs


# channel order in slab: m00 m01 m10 m11 s v0 v1
# stage-1a D-variant scale per channel (folds signs), stage-1b DK scale per channel
CH_DY_SCALE = [-1.0, -1.0, -1.0, -1.0, -1.0, 1.0, 0.5]   # conv_y(DK) scale
CH_DX_SCALE = [-1.0, -1.0, -1.0, -1.0, -1.0, -0.5, 1.0]  # conv_x(DK) scale


def build_graph():
    pack = BandPack()
    sub_ky = band_subs_y(pack, KERN, 1.0)                 # shared smooth-y (also stage 4a)
    sub_kx = band_subs_x(pack, KERN, 1.0, W_IN, W_ALG, OFF1B)    # smooth-x for dy fields
    # dy via raw 3-tap diff of gyk: fold 0.5*CH_DY_SCALE into the 1b Kx band
    sub_kxd = {}
    for sc in sorted(set(0.5 * s for s in CH_DY_SCALE)):
        sub_kxd[sc] = band_subs_x(pack, KERN, sc, W_IN, W_ALG, OFF1B)
    sub_dkx = {}
    for sc in sorted(set(CH_DX_SCALE)):
        sub_dkx[sc] = band_subs_x(pack, DKERN, sc, W_IN, W_ALG, OFF1B)
    sub_kx4 = band_subs_x(pack, KERN, 1.0, W_ALG, XS, RAD)       # stage 4b
    bands_np = pack.packed()
    KTOT = bands_np.shape[1]

    nc = bacc.Bacc()
    x_ext = nc.declare_dram_parameter("x", [7, Y, W_IN], mybir.dt.bfloat16, isOutput=False)
    bands_ext = nc.declare_dram_parameter("bands", [128, KTOT], mybir.dt.bfloat16, isOutput=False)
    mask_ext = nc.declare_dram_parameter("mask", [128, 3], mybir.dt.float32, isOutput=False)
    out_ext = nc.declare_dram_parameter("out", [5, Y, XS], mybir.dt.bfloat16, isOutput=True)

    bf = mybir.dt.bfloat16
    f32 = mybir.dt.float32
    TT = mybir.AluOpType
    ACT_COPY = mybir.ActivationFunctionType.Copy

    with tile.TileContext(nc) as tc:
        with (
            tc.tile_pool(name="const", bufs=1) as constp,
            tc.tile_pool(name="slab", bufs=1) as slabp,
            tc.tile_pool(name="wide", bufs=1) as widep,
            tc.tile_pool(name="gyt", bufs=1) as gytp,
            tc.tile_pool(name="alg", bufs=2) as algp,
            tc.tile_pool(name="pre", bufs=1) as prep,
            tc.tile_pool(name="gyt2", bufs=2) as gyt2p,
            tc.tile_pool(name="outs", bufs=1) as outsp,
            tc.tile_pool(name="psa", bufs=2, space=bass.MemorySpace.PSUM) as psap,
            tc.tile_pool(name="psg", bufs=2, space=bass.MemorySpace.PSUM) as psgp,
        ):
            bands = constp.tile([128, KTOT], bf, tag="bands", name="bands")
            nc.sync.dma_start(bands[:, :], bands_ext[:, :])
            maskt = constp.tile([128, 3], f32, tag="mask", name="mask")
            nc.sync.dma_start(maskt[:, :], mask_ext[:, :])

            # persistent slab: one wide tile per channel [128, YT*W_IN]
            slabw = [slabp.tile([128, YT * W_IN], bf, tag=f"slabw{c}", name=f"slabw{c}")
                     for c in range(7)]
            for c in range(7):
                nc.sync.dma_start(
                    slabw[c][:, :],
                    x_ext[c].rearrange("(t p) x -> p t x", p=128))
            slab = [[slabw[c][:, t * W_IN:(t + 1) * W_IN] for t in range(YT)]
                    for c in range(7)]

            def sview(c):
                """[128, YT, W_ALG] strided view of channel c's alg window."""
                return slabw[c].rearrange("p (t x) -> p t x", x=W_IN)[:, :, OFF1B:OFF1B + W_ALG]

            def wtile(tag):
                return widep.tile([128, WW], bf, tag=tag, name=tag)

            def w3(t_):
                return t_.rearrange("p (t x) -> p t x", x=W_ALG)

            # ---- wide precompute: slab-only dependencies, overlaps input DMA ----
            # coefficient fields (affine in raw s) on ACT
            c1w = wtile("c1w"); c2w = wtile("c2w"); c3w = wtile("c3w"); c4w = wtile("c4w")
            nc.scalar.activation(w3(c1w), sview(4), ACT_COPY, bias=-0.11, scale=0.099)
            nc.scalar.activation(w3(c2w), sview(4), ACT_COPY, bias=0.767, scale=0.055)
            nc.scalar.activation(w3(c3w), sview(4), ACT_COPY, bias=0.732, scale=-0.59)
            nc.scalar.activation(w3(c4w), sview(4), ACT_COPY, bias=0.069, scale=-0.048)
            # trm / u1 / u2 on Pool (idle during DMA/1a)
            trmw = wtile("trmw"); u1w = wtile("u1w"); u2w = wtile("u2w")
            nc.gpsimd.scalar_tensor_tensor(w3(trmw), sview(0), 1.0, sview(3), TT.mult, TT.add)
            nc.gpsimd.scalar_tensor_tensor(w3(u1w), sview(1), 1.0, sview(2), TT.mult, TT.add)
            nc.gpsimd.scalar_tensor_tensor(w3(u2w), sview(3), 1.0, sview(0), TT.mult, TT.subtract)
            # ew = c1 + c3*trm (reuses c3w, c1w);  Ccw = c4*trm (reuses c4w)
            nc.vector.tensor_tensor(c3w[:, :], c3w[:, :], trmw[:, :], TT.mult)
            nc.vector.tensor_tensor(c1w[:, :], c1w[:, :], c3w[:, :], TT.add)
            nc.vector.tensor_tensor(c4w[:, :], c4w[:, :], trmw[:, :], TT.mult)
            ew, Ccw = c1w, c4w

            def conv_group(psum_ap, subs, lhsT_fn):
                n = len(subs)
                for i, (k, a, b, off, rows) in enumerate(subs):
                    nc.tensor.matmul(
                        psum_ap[:, a:b],
                        lhsT_fn(k, rows),
                        bands[:rows, off:off + b - a],
                        start=(i == 0),
                        stop=(i == n - 1),
                    )

            # ---------------- stage 1a: conv_y (Ky only) for all channels ----------------
            # dy rides a raw circular 3-tap diff of gyk along the free y dim;
            # the 0.5 and per-channel signs are folded into the 1b Kx bands.
            gyk_all, gyd_all = [], []
            n1a = 0
            for c in range(7):
                gyk = [gytp.tile([128, 1024], bf, tag=f"gyk{c}_{xt}", name=f"gyk{c}_{xt}") for xt in range(3)]
                gyd = [gytp.tile([128, 1024], bf, tag=f"gyd{c}_{xt}", name=f"gyd{c}_{xt}") for xt in range(3)]
                gyk_all.append(gyk); gyd_all.append(gyd)
                for xt, (x0, xw) in enumerate(XT_IN):
                    ps = psap.tile([128, 2 * HALF], f32, tag="psa", name="psa")
                    for h in range(2):
                        conv_group(
                            ps[:xw, h * HALF:(h + 1) * HALF], sub_ky[h],
                            lambda k, rows: slab[c][k][:, x0:x0 + xw])
                    # evac split ACT(2/3) / DVE(1/3)
                    if n1a % 3 == 2:
                        nc.vector.tensor_copy(gyk[xt][:xw, :], ps[:xw, :])
                    else:
                        nc.scalar.copy(gyk[xt][:xw, :], ps[:xw, :])
                    g, d = gyk[xt], gyd[xt]
                    # interior diff: mostly DVE, a few on Pool
                    if n1a % 5 == 4:
                        nc.gpsimd.scalar_tensor_tensor(
                            d[:xw, 1:1023], g[:xw, 2:1024], 1.0, g[:xw, 0:1022],
                            TT.mult, TT.subtract)
                    else:
                        nc.vector.tensor_tensor(d[:xw, 1:1023], g[:xw, 2:1024], g[:xw, 0:1022], TT.subtract)
                    # merged circular boundary fixup (cols 0 and 1023) on DVE
                    nc.vector.tensor_tensor(
                        d[:xw, 0:1024:1023], g[:xw, 1::-1], g[:xw, 1023:1021:-1],
                        TT.subtract)
                    n1a += 1

            # ------- stage 1b + algebra, y-tile major -------
            pre = [[prep.tile([128, W_ALG], bf, tag=f"pre{f}_{t}", name=f"pre{f}_{t}") for t in range(YT)]
                   for f in range(5)]
            A = slice(OFF1B, OFF1B + W_ALG)
            dxsubs = {c: sub_dkx[CH_DX_SCALE[c]] for c in range(7)}

            def _emit(ps_view, t, specs):
                first = True
                for gi, (gt, subs) in enumerate(specs):
                    n = len(subs)
                    for i, (k, a, b, off, rows) in enumerate(subs):
                        nc.tensor.matmul(
                            ps_view[:, a:b],
                            gt[k][:rows, t * 128:(t + 1) * 128],
                            bands[:rows, off:off + b - a],
                            start=first,
                            stop=(gi == len(specs) - 1 and i == n - 1),
                        )
                        first = False

            def grad_pair(t, specsA, specsB):
                ps = psgp.tile([128, 2 * HALF], f32, tag="pg", name="pg")
                _emit(ps[:, 0:HALF], t, specsA)
                _emit(ps[:, HALF:2 * HALF], t, specsB)
                return ps

            def pair_evac(ps, dst):
                """One ACT copy of both banks' [:, :W_ALG] into dst [128, 2*W_ALG]."""
                srcv = ps.rearrange("p (b x) -> p b x", b=2)[:, :, 0:W_ALG]
                dstv = dst.rearrange("p (b x) -> p b x", x=W_ALG)
                nc.scalar.copy(dstv, srcv)

            for t in range(YT):
                m0a, m1a = slab[0][t][:, A], slab[1][t][:, A]
                m2a, m3a = slab[2][t][:, A], slab[3][t][:, A]
                v0a, v1a = slab[5][t][:, A], slab[6][t][:, A]
                TS = slice(t * W_ALG, (t + 1) * W_ALG)

                def tmp(tag):
                    return algp.tile([128, W_ALG], bf, tag=tag, name=tag)

                # w and trE share one 2-bank psum; evac by ACT, then SBUF algebra
                ps_wt = grad_pair(
                    t,
                    [(gyk_all[5], dxsubs[5]), (gyd_all[6], sub_kxd[0.5 * CH_DY_SCALE[6]])],
                    [(gyd_all[5], sub_kxd[0.5 * CH_DY_SCALE[5]]), (gyk_all[6], dxsubs[6])])
                wtf = algp.tile([128, 2 * W_ALG], bf, tag="wtf", name="wtf")
                pair_evac(ps_wt, wtf)
                wu1 = tmp("wu1"); wu2 = tmp("wu2")
                nc.vector.tensor_tensor(wu1[:, :], wtf[:, 0:W_ALG], u1w[:, TS], TT.mult)
                nc.vector.tensor_tensor(wu2[:, :], wtf[:, 0:W_ALG], u2w[:, TS], TT.mult)
                t1 = tmp("t1"); Ac = tmp("Ac")
                nc.vector.tensor_tensor(t1[:, :], wtf[:, W_ALG:2 * W_ALG], c2w[:, TS], TT.mult)
                nc.vector.tensor_tensor(Ac[:, :], t1[:, :], ew[:, TS], TT.add)

                # sdot (folds: psum = -dys, -dxs)
                ps_s = grad_pair(t, [(gyd_all[4], sub_kxd[0.5 * CH_DY_SCALE[4]])],
                                 [(gyk_all[4], dxsubs[4])])
                gs = algp.tile([128, 2 * W_ALG], bf, tag="gs", name="gs")
                pair_evac(ps_s, gs)
                sd1 = tmp("q1"); sd2 = tmp("q2")
                nc.vector.tensor_tensor(sd1[:, :], gs[:, 0:W_ALG], v0a, TT.mult)
                nc.vector.tensor_tensor(sd2[:, :], gs[:, W_ALG:2 * W_ALG], v1a, TT.mult)
                nc.vector.tensor_tensor(pre[4][t][:, :], sd1[:, :], sd2[:, :], TT.add)

                mas = (m0a, m1a, m2a, m3a)
                for ch in range(4):
                    ps_ab = grad_pair(t, [(gyd_all[ch], sub_kxd[0.5 * CH_DY_SCALE[ch]])],
                                      [(gyk_all[ch], dxsubs[ch])])
                    q1 = tmp("q1"); q2 = tmp("q2"); r = tmp("r"); b = tmp("b")
                    # products: q1 from psum on DVE, q2 from psum on Pool-STT
                    nc.vector.tensor_tensor(q1[:, :], ps_ab[:, 0:W_ALG], v0a, TT.mult)
                    nc.gpsimd.scalar_tensor_tensor(
                        q2[:, :], ps_ab[:, HALF:HALF + W_ALG], 1.0, v1a, TT.mult, TT.mult)
                    nc.vector.tensor_tensor(r[:, :], Ac[:, :], mas[ch], TT.mult)
                    # off-chain combine on Pool: b = r +- wu (+ Cc for ch0)
                    if ch == 0:
                        nc.gpsimd.scalar_tensor_tensor(
                            b[:, :], r[:, :], 1.0, wu1[:, :], TT.mult, TT.subtract)
                        nc.gpsimd.scalar_tensor_tensor(
                            b[:, :], b[:, :], 1.0, Ccw[:, TS], TT.mult, TT.add)
                    elif ch == 3:
                        nc.gpsimd.scalar_tensor_tensor(
                            b[:, :], r[:, :], 1.0, wu1[:, :], TT.mult, TT.add)
                    else:
                        nc.gpsimd.scalar_tensor_tensor(
                            b[:, :], r[:, :], 1.0, wu2[:, :], TT.mult, TT.subtract)
                    nc.vector.tensor_tensor(q1[:, :], q1[:, :], q2[:, :], TT.add)
                    nc.vector.tensor_tensor(pre[ch][t][:, :], q1[:, :], b[:, :], TT.add)

            # ---------------- stage 4: final smooth of 5 fields ----------------
            n4a = 0
            n4b = 0
            for f in range(5):
                gy2 = [gyt2p.tile([128, 1024], bf, tag=f"gy2{xt}", name=f"gy2{xt}") for xt in range(3)]
                for xt, (x0, xw) in enumerate(XT_ALG):
                    ps = psap.tile([128, 2 * HALF], f32, tag="psa", name="psa")
                    for h in range(2):
                        conv_group(
                            ps[:xw, h * HALF:(h + 1) * HALF], sub_ky[h],
                            lambda k, rows: pre[f][k][:, x0:x0 + xw])
                    # mask-scaled single-instruction evac, round-robin engine
                    sc = maskt[:xw, xt:xt + 1]
                    if n4a % 3 == 0:
                        nc.scalar.activation(gy2[xt][:xw, :], ps[:xw, :], ACT_COPY, scale=sc)
                    elif n4a % 3 == 1:
                        nc.vector.tensor_scalar(gy2[xt][:xw, :], ps[:xw, :], sc, None, TT.mult)
                    else:
                        nc.gpsimd.tensor_scalar(gy2[xt][:xw, :], ps[:xw, :], sc, None, TT.mult)
                    n4a += 1
                ow = outsp.tile([128, YT * XS], bf, tag="ow", name="ow")
                for tp in range(YT // 2):
                    ps = psap.tile([128, 2 * HALF], f32, tag="psa", name="psa")
                    for half in range(2):
                        t = 2 * tp + half
                        conv_group(
                            ps[:, half * HALF:half * HALF + XS], sub_kx4,
                            lambda k, rows: gy2[k][:rows, t * 128:(t + 1) * 128])
                    dst = ow.rearrange("p (t x) -> p t x", x=XS)[:, 2 * tp:2 * tp + 2, :]
                    src = ps.rearrange("p (t x) -> p t x", x=HALF)[:, :, 0:XS]
                    if n4b % 4 == 0:
                        nc.vector.tensor_copy(dst, src)
                    elif n4b % 4 == 2:
                        nc.gpsimd.tensor_copy(dst, src)
                    else:
                        nc.scalar.copy(dst, src)
                    n4b += 1
                nc.sync.dma_start(
                    out_ext[f].rearrange("(t p) x -> p t x", p=128), ow[:, :])

    nc.compile()
    return nc, bands_np


_CACHE = {}


def _get_graph():
    if "nc" not in _CACHE:
        _CACHE["nc"], _CACHE["bands"] = build_graph()
    return _CACHE["nc"], _CACHE["bands"]


def host_prep(y, v):
    m = y[:4]
    s = y[4:5]
    v_lr = v[:, ::-1, :].copy()
    v_lr[0] *= -1.0
    vs = 0.5 * (v + v_lr)
    f = np.concatenate([m, s, vs], axis=0).astype(F32)      # [7, Y, X]
    fp = np.pad(f, ((0, 0), (0, 0), (H, H)), mode='edge')
    slabs, masks = [], []
    for c in range(NCORES):
        x0 = c * XS
        slabs.append(np.ascontiguousarray(fp[:, :, x0:x0 + W_IN]).astype(BF16))
        g = x0 + np.arange(W_ALG) - RAD
        mk = ((g >= AP_CUT) & (g < X - AP_CUT)).astype(F32)
        mk_t = np.zeros((128, 3), dtype=F32)
        for xt, (a, w) in enumerate(XT_ALG):
            mk_t[:w, xt] = mk[a:a + w]
        masks.append(mk_t)
    return slabs, masks


def kernel(y, v):
    y = np.asarray(y, dtype=F32)
    v = np.asarray(v, dtype=F32)
    nc, bands_np = _get_graph()
    slabs, masks = host_prep(y, v)
    in_maps = [
        {"x": slabs[c], "bands": bands_np, "mask": masks[c]}
        for c in range(NCORES)
    ]
    res = run_bass_kernel_spmd(nc, in_maps, core_ids=list(range(NCORES)))
    out = np.concatenate([res.results[c]["out"] for c in range(NCORES)], axis=2)
    return out.astype(F32)


# revision 8
# speedup vs baseline: 1.1064x; 1.0675x over previous
"""Distributed Trainium2 Bass kernel for nn_ClosedFlyLoop.

Strategy (8 NeuronCores, shard X into 8 blocks of 256):
 - host: symmetrize v, split y into (m[4], s), pad X edge-replicate by H=25,
   cut per-core overlapping slabs [7, 1024, 306] (halo covers smooth+diff+
   post-smooth supports; the AP_CUT zeroing erases the only global-edge
   discrepancy), convert to bf16.
 - device, per core (no collectives needed):
     wide precompute: everything that depends only on the raw slab (the four
               affine-in-s coefficient fields, trm/u1/u2, their combinations)
               is computed once for the whole core as [128, 8*280] strided
               ops, overlapped with the input DMA.
     stage 1a: conv along y (circular) as banded matmuls that also transpose
               layout [y,x] -> [x,y']  (lhsT = data chunk, rhs = Toeplitz
               band); 2-bank [128,1024] psum groups, single-instruction
               evacs split ACT/DVE, dy via circular 3-tap diff split DVE/Pool
     stage 1b: conv along x as banded matmuls transposing back [x,y]->[y,x'],
               producing gradient pairs in PSUM. w/trE and s pairs are
               evacuated by ACT; the four m pairs are consumed straight from
               PSUM (q1 on DVE, q2 on Pool-STT). Adds are tree-structured:
               Pool combines b = r +- w*u off the critical chain.
     stage 4a/4b: final Gaussian smooth of the 5 masked pre-fields, 2-bank
               psum groups, mask folded into the 4a evac as a per-partition
               scalar multiply, 4b outputs packed 2 y-tiles per bank; evacs
               round-robin ACT/DVE/Pool.
 - host: concatenate per-core [5, 1024, 256] bf16 outputs along X, upcast.
"""
import numpy as np
import ml_dtypes

import concourse.bass as bass
import concourse.bacc as bacc
import concourse.mybir as mybir
from concourse import tile
from concourse.bass_utils import run_bass_kernel_spmd

BF16 = ml_dtypes.bfloat16
F32 = np.float32

Y, X = 1024, 2048
NCORES = 8
XS = X // NCORES            # 256
RAD = 12                    # gauss radius: int(4.0*3.0+0.5)
H = 2 * RAD + 1             # 25
W_IN = XS + 2 * H           # 306
W_ALG = XS + 2 * RAD        # 280
OFF1B = H - RAD             # 13: slab-coord offset of alg window
AP_CUT = 15
YT = Y // 128               # 8 y tiles
XT_IN = [(0, 128), (128, 128), (256, W_IN - 256)]       # x tiles of slab (128,128,50)
XT_ALG = [(0, 128), (128, 128), (256, W_ALG - 256)]     # x tiles of alg width (128,128,24)
HALF = 512                  # y' half width for stage a psum
WW = YT * W_ALG             # 2240: wide-op width


def _gauss():
    r = RAD
    x = np.arange(-r, r + 1, dtype=np.float64)
    k = np.exp(-0.5 * (x / 3.0) ** 2)
    k = (k / k.sum()).astype(np.float64)
    dk = np.convolve(k, [-0.5, 0.0, 0.5])
    return k.astype(F32), dk.astype(F32)


KERN, DKERN = _gauss()      # 25 taps (rad 12), 27 taps (rad 13)


# ---------------- band submatrix machinery (host) ----------------
class BandPack:
    """Dedup + pack all band submatrices into one [128, K] bf16 constant."""

    def __init__(self):
        self.blocks = {}
        self.cols = []
        self.total = 0

    def add(self, sub):
        sub16 = np.ascontiguousarray(sub.astype(BF16))
        key = (sub16.shape, sub16.tobytes())
        if key not in self.blocks:
            pad = np.zeros((128, sub16.shape[1]), dtype=BF16)
            pad[: sub16.shape[0]] = sub16
            self.blocks[key] = self.total
            self.cols.append(pad)
            self.total += sub16.shape[1]
        return self.blocks[key]

    def packed(self):
        return np.concatenate(self.cols, axis=1)


def band_subs_y(pack, ker, scale):
    """Circular conv along y (1024). Returns per half h: list of
    (chunk k, col a, col b, packed offset, rows) for rhs = B[krows, h*512+a : h*512+b]."""
    r = ker.shape[0] // 2
    B = np.zeros((Y, Y), dtype=F32)
    for j in range(Y):
        for t in range(-r, r + 1):
            B[(j + t) % Y, j] = ker[r + t] * scale
    out = []
    for h in range(2):
        subs = []
        for k in range(YT):
            sub = B[k * 128:(k + 1) * 128, h * HALF:(h + 1) * HALF]
            cols = np.flatnonzero(np.any(sub != 0.0, axis=0))
            if cols.size == 0:
                continue
            a, b = int(cols[0]), int(cols[-1] + 1)
            assert b - a == cols.size
            off = pack.add(sub[:, a:b])
            subs.append((k, a, b, off, 128))
        out.append(subs)
    return out


def band_subs_x(pack, ker, scale, n_in, n_out, off_in):
    """conv along x: out[j] = sum_t kc[t] in[j + off_in + t].
    Returns list of (chunk k, col a, col b, packed offset, rows)."""
    r = ker.shape[0] // 2
    B = np.zeros((n_in, n_out), dtype=F32)
    for j in range(n_out):
        for t in range(-r, r + 1):
            i = j + off_in + t
            if 0 <= i < n_in:
                B[i, j] = ker[r + t] * scale
    subs = []
    nchunks = (n_in + 127) // 128
    for k in range(nchunks):
        rows = min(128, n_in - k * 128)
        sub = B[k * 128:k * 128 + rows, :]
        cols = np.flatnonzero(np.any(sub != 0.0, axis=0))
        if cols.size == 0:
            continue
        a, b = int(cols[0]), int(cols[-1] + 1)
        assert b - a == cols.size
        off = pack.add(sub[:, a:b])
        subs.append((k, a, b, off, rows))
    return subs


# channel order in slab: m00 m01 m10 m11 s v0 v1
# stage-1a D-variant scale per channel (folds signs), stage-1b DK scale per channel
CH_DY_SCALE = [-1.0, -1.0, -1.0, -1.0, -1.0, 1.0, 0.5]   # conv_y(DK) scale
CH_DX_SCALE = [-1.0, -1.0, -1.0, -1.0, -1.0, -0.5, 1.0]  # conv_x(DK) scale


def build_graph():
    pack = BandPack()
    sub_ky = band_subs_y(pack, KERN, 1.0)                 # shared smooth-y (also stage 4a)
    sub_kx = band_subs_x(pack, KERN, 1.0, W_IN, W_ALG, OFF1B)    # smooth-x for dy fields
    # dy via raw 3-tap diff of gyk: fold 0.5*CH_DY_SCALE into the 1b Kx band
    sub_kxd = {}
    for sc in sorted(set(0.5 * s for s in CH_DY_SCALE)):
        sub_kxd[sc] = band_subs_x(pack, KERN, sc, W_IN, W_ALG, OFF1B)
    sub_dkx = {}
    for sc in sorted(set(CH_DX_SCALE)):
        sub_dkx[sc] = band_subs_x(pack, DKERN, sc, W_IN, W_ALG, OFF1B)
    sub_kx4 = band_subs_x(pack, KERN, 1.0, W_ALG, XS, RAD)       # stage 4b
    bands_np = pack.packed()
    KTOT = bands_np.shape[1]

    nc = bacc.Bacc()
    x_ext = nc.declare_dram_parameter("x", [7, Y, W_IN], mybir.dt.bfloat16, isOutput=False)
    bands_ext = nc.declare_dram_parameter("bands", [128, KTOT], mybir.dt.bfloat16, isOutput=False)
    mask_ext = nc.declare_dram_parameter("mask", [128, 3], mybir.dt.float32, isOutput=False)
    out_ext = nc.declare_dram_parameter("out", [5, Y, XS], mybir.dt.bfloat16, isOutput=True)

    bf = mybir.dt.bfloat16
    f32 = mybir.dt.float32
    TT = mybir.AluOpType
    ACT_COPY = mybir.ActivationFunctionType.Copy

    with tile.TileContext(nc) as tc:
        with (
            tc.tile_pool(name="const", bufs=1) as constp,
            tc.tile_pool(name="slab", bufs=1) as slabp,
            tc.tile_pool(name="wide", bufs=1) as widep,
            tc.tile_pool(name="gyt", bufs=1) as gytp,
            tc.tile_pool(name="alg", bufs=2) as algp,
            tc.tile_pool(name="pre", bufs=1) as prep,
            tc.tile_pool(name="gyt2", bufs=2) as gyt2p,
            tc.tile_pool(name="outs", bufs=1) as outsp,
            tc.tile_pool(name="psa", bufs=2, space=bass.MemorySpace.PSUM) as psap,
            tc.tile_pool(name="psg", bufs=2, space=bass.MemorySpace.PSUM) as psgp,
        ):
            bands = constp.tile([128, KTOT], bf, tag="bands", name="bands")
            nc.sync.dma_start(bands[:, :], bands_ext[:, :])
            maskt = constp.tile([128, 3], f32, tag="mask", name="mask")
            nc.sync.dma_start(maskt[:, :], mask_ext[:, :])

            # persistent slab: one wide tile per channel [128, YT*W_IN]
            slabw = [slabp.tile([128, YT * W_IN], bf, tag=f"slabw{c}", name=f"slabw{c}")
                     for c in range(7)]
            for c in range(7):
                nc.sync.dma_start(
                    slabw[c][:, :],
                    x_ext[c].rearrange("(t p) x -> p t x", p=128))
            slab = [[slabw[c][:, t * W_IN:(t + 1) * W_IN] for t in range(YT)]
                    for c in range(7)]

            def sview(c):
                """[128, YT, W_ALG] strided view of channel c's alg window."""
                return slabw[c].rearrange("p (t x) -> p t x", x=W_IN)[:, :, OFF1B:OFF1B + W_ALG]

            def wtile(tag):
                return widep.tile([128, WW], bf, tag=tag, name=tag)

            def w3(t_):
                return t_.rearrange("p (t x) -> p t x", x=W_ALG)

            # ---- wide precompute: slab-only dependencies, overlaps input DMA ----
            # coefficient fields (affine in raw s) on ACT
            c1w = wtile("c1w"); c2w = wtile("c2w"); c3w = wtile("c3w"); c4w = wtile("c4w")
            nc.scalar.activation(w3(c1w), sview(4), ACT_COPY, bias=-0.11, scale=0.099)
            nc.scalar.activation(w3(c2w), sview(4), ACT_COPY, bias=0.767, scale=0.055)
            nc.scalar.activation(w3(c3w), sview(4), ACT_COPY, bias=0.732, scale=-0.59)
            nc.scalar.activation(w3(c4w), sview(4), ACT_COPY, bias=0.069, scale=-0.048)
            # trm / u1 / u2 on Pool (idle during DMA/1a)
            trmw = wtile("trmw"); u1w = wtile("u1w"); u2w = wtile("u2w")
            nc.gpsimd.scalar_tensor_tensor(w3(trmw), sview(0), 1.0, sview(3), TT.mult, TT.add)
            nc.gpsimd.scalar_tensor_tensor(w3(u1w), sview(1), 1.0, sview(2), TT.mult, TT.add)
            nc.gpsimd.scalar_tensor_tensor(w3(u2w), sview(3), 1.0, sview(0), TT.mult, TT.subtract)
            # ew = c1 + c3*trm (reuses c3w, c1w);  Ccw = c4*trm (reuses c4w)
            nc.vector.tensor_tensor(c3w[:, :], c3w[:, :], trmw[:, :], TT.mult)
            nc.vector.tensor_tensor(c1w[:, :], c1w[:, :], c3w[:, :], TT.add)
            nc.vector.tensor_tensor(c4w[:, :], c4w[:, :], trmw[:, :], TT.mult)
            ew, Ccw = c1w, c4w

            def conv_group(psum_ap, subs, lhsT_fn):
                n = len(subs)
                for i, (k, a, b, off, rows) in enumerate(subs):
                    nc.tensor.matmul(
                        psum_ap[:, a:b],
                        lhsT_fn(k, rows),
                        bands[:rows, off:off + b - a],
                        start=(i == 0),
                        stop=(i == n - 1),
                    )

            # ---------------- stage 1a: conv_y (Ky only) for all channels ----------------
            # dy rides a raw circular 3-tap diff of gyk along the free y dim;
            # the 0.5 and per-channel signs are folded into the 1b Kx bands.
            gyk_all, gyd_all = [], []
            n1a = 0
            for c in range(7):
                gyk = [gytp.tile([128, 1024], bf, tag=f"gyk{c}_{xt}", name=f"gyk{c}_{xt}") for xt in range(3)]
                gyd = [gytp.tile([128, 1024], bf, tag=f"gyd{c}_{xt}", name=f"gyd{c}_{xt}") for xt in range(3)]
                gyk_all.append(gyk); gyd_all.append(gyd)
                for xt, (x0, xw) in enumerate(XT_IN):
                    ps = psap.tile([128, 2 * HALF], f32, tag="psa", name="psa")
                    for h in range(2):
                        conv_group(
                            ps[:xw, h * HALF:(h + 1) * HALF], sub_ky[h],
                            lambda k, rows: slab[c][k][:, x0:x0 + xw])
                    # evac split ACT(2/3) / DVE(1/3)
                    if n1a % 3 == 2:
                        nc.vector.tensor_copy(gyk[xt][:xw, :], ps[:xw, :])
                    else:
                        nc.scalar.copy(gyk[xt][:xw, :], ps[:xw, :])
                    g, d = gyk[xt], gyd[xt]
                    # interior diff: mostly DVE, a few on Pool
                    if n1a % 5 == 4:
                        nc.gpsimd.scalar_tensor_tensor(
                            d[:xw, 1:1023], g[:xw, 2:1024], 1.0, g[:xw, 0:1022],
                            TT.mult, TT.subtract)
                    else:
                        nc.vector.tensor_tensor(d[:xw, 1:1023], g[:xw, 2:1024], g[:xw, 0:1022], TT.subtract)
                    # merged circular boundary fixup (cols 0 and 1023) on DVE
                    nc.vector.tensor_tensor(
                        d[:xw, 0:1024:1023], g[:xw, 1::-1], g[:xw, 1023:1021:-1],
                        TT.subtract)
                    n1a += 1

            # ------- stage 1b + algebra, y-tile major -------
            pre = [[prep.tile([128, W_ALG], bf, tag=f"pre{f}_{t}", name=f"pre{f}_{t}") for t in range(YT)]
                   for f in range(5)]
            A = slice(OFF1B, OFF1B + W_ALG)
            dxsubs = {c: sub_dkx[CH_DX_SCALE[c]] for c in range(7)}

            def _emit(ps_view, t, specs):
                first = True
                for gi, (gt, subs) in enumerate(specs):
                    n = len(subs)
                    for i, (k, a, b, off, rows) in enumerate(subs):
                        nc.tensor.matmul(
                            ps_view[:, a:b],
                            gt[k][:rows, t * 128:(t + 1) * 128],
                            bands[:rows, off:off + b - a],
                            start=first,
                            stop=(gi == len(specs) - 1 and i == n - 1),
                        )
                        first = False

            def grad_pair(t, specsA, specsB):
                ps = psgp.tile([128, 2 * HALF], f32, tag="pg", name="pg")
                _emit(ps[:, 0:HALF], t, specsA)
                _emit(ps[:, HALF:2 * HALF], t, specsB)
                return ps

            def pair_evac(ps, dst):
                """One ACT copy of both banks' [:, :W_ALG] into dst [128, 2*W_ALG]."""
                srcv = ps.rearrange("p (b x) -> p b x", b=2)[:, :, 0:W_ALG]
                dstv = dst.rearrange("p (b x) -> p b x", x=W_ALG)
                nc.scalar.copy(dstv, srcv)

            for t in range(YT):
                m0a, m1a = slab[0][t][:, A], slab[1][t][:, A]
                m2a, m3a = slab[2][t][:, A], slab[3][t][:, A]
                v0a, v1a = slab[5][t][:, A], slab[6][t][:, A]
                TS = slice(t * W_ALG, (t + 1) * W_ALG)

                def tmp(tag):
                    return algp.tile([128, W_ALG], bf, tag=tag, name=tag)

                # w and trE share one 2-bank psum; evac by ACT, then SBUF algebra
                ps_wt = grad_pair(
                    t,
                    [(gyk_all[5], dxsubs[5]), (gyd_all[6], sub_kxd[0.5 * CH_DY_SCALE[6]])],
                    [(gyd_all[5], sub_kxd[0.5 * CH_DY_SCALE[5]]), (gyk_all[6], dxsubs[6])])
                wtf = algp.tile([128, 2 * W_ALG], bf, tag="wtf", name="wtf")
                pair_evac(ps_wt, wtf)
                wu1 = tmp("wu1"); wu2 = tmp("wu2")
                nc.vector.tensor_tensor(wu1[:, :], wtf[:, 0:W_ALG], u1w[:, TS], TT.mult)
                nc.vector.tensor_tensor(wu2[:, :], wtf[:, 0:W_ALG], u2w[:, TS], TT.mult)
                t1 = tmp("t1"); Ac = tmp("Ac")
                nc.vector.tensor_tensor(t1[:, :], wtf[:, W_ALG:2 * W_ALG], c2w[:, TS], TT.mult)
                nc.vector.tensor_tensor(Ac[:, :], t1[:, :], ew[:, TS], TT.add)

                # sdot (folds: psum = -dys, -dxs)
                ps_s = grad_pair(t, [(gyd_all[4], sub_kxd[0.5 * CH_DY_SCALE[4]])],
                                 [(gyk_all[4], dxsubs[4])])
                gs = algp.tile([128, 2 * W_ALG], bf, tag="gs", name="gs")
                pair_evac(ps_s, gs)
                sd1 = tmp("q1"); sd2 = tmp("q2")
                nc.vector.tensor_tensor(sd1[:, :], gs[:, 0:W_ALG], v0a, TT.mult)
                nc.vector.tensor_tensor(sd2[:, :], gs[:, W_ALG:2 * W_ALG], v1a, TT.mult)
                nc.vector.tensor_tensor(pre[4][t][:, :], sd1[:, :], sd2[:, :], TT.add)

                mas = (m0a, m1a, m2a, m3a)
                for ch in range(4):
                    ps_ab = grad_pair(t, [(gyd_all[ch], sub_kxd[0.5 * CH_DY_SCALE[ch]])],
                                      [(gyk_all[ch], dxsubs[ch])])
                    q1 = tmp("q1"); q2 = tmp("q2"); r = tmp("r"); b = tmp("b")
                    # products: q1 from psum on DVE, q2 from psum on Pool-STT
                    nc.vector.tensor_tensor(q1[:, :], ps_ab[:, 0:W_ALG], v0a, TT.mult)
                    nc.gpsimd.scalar_tensor_tensor(
                        q2[:, :], ps_ab[:, HALF:HALF + W_ALG], 1.0, v1a, TT.mult, TT.mult)
                    nc.vector.tensor_tensor(r[:, :], Ac[:, :], mas[ch], TT.mult)
                    # off-chain combine on Pool: b = r +- wu (+ Cc for ch0)
                    if ch == 0:
                        nc.gpsimd.scalar_tensor_tensor(
                            b[:, :], r[:, :], 1.0, wu1[:, :], TT.mult, TT.subtract)
                        nc.gpsimd.scalar_tensor_tensor(
                            b[:, :], b[:, :], 1.0, Ccw[:, TS], TT.mult, TT.add)
                    elif ch == 3:
                        nc.gpsimd.scalar_tensor_tensor(
                            b[:, :], r[:, :], 1.0, wu1[:, :], TT.mult, TT.add)
                    else:
                        nc.gpsimd.scalar_tensor_tensor(
                            b[:, :], r[:, :], 1.0, wu2[:, :], TT.mult, TT.subtract)
                    nc.vector.tensor_tensor(q1[:, :], q1[:, :], q2[:, :], TT.add)
                    nc.vector.tensor_tensor(pre[ch][t][:, :], q1[:, :], b[:, :], TT.add)

            # ---------------- stage 4: final smooth of 5 fields ----------------
            # 4a on psa pool, 4b on psg pool (idle after 1b) so they do not
            # contend; gy2 kept as per-(xt,half) [128,512] tiles so 4b of the
            # first y-half starts before the second half is smoothed; output
            # DMA fires per y-half of each field.
            n4a = 0
            n4b = 0
            for f in range(5):
                gy2 = [[gyt2p.tile([128, HALF], bf, tag=f"gy2{xt}_{h}", name=f"gy2{xt}_{h}")
                        for h in range(2)] for xt in range(3)]
                for xt, (x0, xw) in enumerate(XT_ALG):
                    ps = psap.tile([128, 2 * HALF], f32, tag="psa", name="psa")
                    sc = maskt[:xw, xt:xt + 1]
                    for h in range(2):
                        conv_group(
                            ps[:xw, h * HALF:(h + 1) * HALF], sub_ky[h],
                            lambda k, rows: pre[f][k][:, x0:x0 + xw])
                        # mask-scaled evac per half, round-robin engine
                        dst = gy2[xt][h][:xw, :]
                        srcv = ps[:xw, h * HALF:(h + 1) * HALF]
                        if n4a % 3 == 0:
                            nc.scalar.activation(dst, srcv, ACT_COPY, scale=sc)
                        elif n4a % 3 == 1:
                            nc.vector.tensor_scalar(dst, srcv, sc, None, TT.mult)
                        else:
                            nc.gpsimd.tensor_scalar(dst, srcv, sc, None, TT.mult)
                        n4a += 1
                for hh in range(2):
                    ow = outsp.tile([128, 4 * XS], bf, tag=f"ow{hh}", name=f"ow{hh}")
                    for tp in range(2):
                        ps = psgp.tile([128, 2 * HALF], f32, tag="pg", name="pg")
                        for half in range(2):
                            t = 4 * hh + 2 * tp + half
                            conv_group(
                                ps[:, half * HALF:half * HALF + XS], sub_kx4,
                                lambda k, rows: gy2[k][t // 4][:rows, (t % 4) * 128:(t % 4 + 1) * 128])
                        dst = ow.rearrange("p (t x) -> p t x", x=XS)[:, 2 * tp:2 * tp + 2, :]
                        src = ps.rearrange("p (t x) -> p t x", x=HALF)[:, :, 0:XS]
                        if n4b % 4 == 0:
                            nc.vector.tensor_copy(dst, src)
                        elif n4b % 4 == 2:
                            nc.gpsimd.tensor_copy(dst, src)
                        else:
                            nc.scalar.copy(dst, src)
                        n4b += 1
                    nc.sync.dma_start(
                        out_ext[f, hh * HALF:(hh + 1) * HALF, :].rearrange(
                            "(t p) x -> p t x", p=128),
                        ow[:, :])

    nc.compile()
    return nc, bands_np


_CACHE = {}


def _get_graph():
    if "nc" not in _CACHE:
        _CACHE["nc"], _CACHE["bands"] = build_graph()
    return _CACHE["nc"], _CACHE["bands"]


def host_prep(y, v):
    m = y[:4]
    s = y[4:5]
    v_lr = v[:, ::-1, :].copy()
    v_lr[0] *= -1.0
    vs = 0.5 * (v + v_lr)
    f = np.concatenate([m, s, vs], axis=0).astype(F32)      # [7, Y, X]
    fp = np.pad(f, ((0, 0), (0, 0), (H, H)), mode='edge')
    slabs, masks = [], []
    for c in range(NCORES):
        x0 = c * XS
        slabs.append(np.ascontiguousarray(fp[:, :, x0:x0 + W_IN]).astype(BF16))
        g = x0 + np.arange(W_ALG) - RAD
        mk = ((g >= AP_CUT) & (g < X - AP_CUT)).astype(F32)
        mk_t = np.zeros((128, 3), dtype=F32)
        for xt, (a, w) in enumerate(XT_ALG):
            mk_t[:w, xt] = mk[a:a + w]
        masks.append(mk_t)
    return slabs, masks


def kernel(y, v):
    y = np.asarray(y, dtype=F32)
    v = np.asarray(v, dtype=F32)
    nc, bands_np = _get_graph()
    slabs, masks = host_prep(y, v)
    in_maps = [
        {"x": slabs[c], "bands": bands_np, "mask": masks[c]}
        for c in range(NCORES)
    ]
    res = run_bass_kernel_spmd(nc, in_maps, core_ids=list(range(NCORES)))
    out = np.concatenate([res.results[c]["out"] for c in range(NCORES)], axis=2)
    return out.astype(F32)


# revision 11
# speedup vs baseline: 1.1158x; 1.0085x over previous
"""Distributed Trainium2 Bass kernel for nn_ClosedFlyLoop.

Strategy (8 NeuronCores, shard X into 8 blocks of 256):
 - host: symmetrize v, split y into (m[4], s), pad X edge-replicate by H=25,
   cut per-core overlapping slabs [7, 1024, 306] (halo covers smooth+diff+
   post-smooth supports; the AP_CUT zeroing erases the only global-edge
   discrepancy), convert to bf16.
 - device, per core (no collectives needed):
     input DMA ordered (s, v0, v1, m...) so the coefficient/velocity wide
       precomputes and the s/v conv chains start inside the DMA window.
     stage 1a: conv along y (circular) as banded matmuls that also transpose
       layout [y,x] -> [x,y']; 2-bank [128,1024] psum groups with single ACT
       evacs; dy via circular 3-tap diff (DVE, overflow to Pool).
     w/trE prologue: the vorticity/strain pairs for all 8 y-tiles run right
       after the v channels' 1a, evac into one wide tile; wu1/wu2/t1/Ac are
       then single [128, 8*280] wide ops (buffers reused in place).
     s prologue: sdot products straight from PSUM into a wide pre field.
     m-loop (channel-major): per channel a wide b = Ac*m +- w*u field on
       Pool, then per y-tile the PSUM gradient pair is consumed directly
       (q1 DVE / q2 Pool-STT) and folded with two adds.
     stage 4a/4b: final smooth; 4a on psa pool, 4b on psg pool, per-half gy2
       tiles, mask folded into 4a evacs, outputs packed and DMAd per y-half.
 - host: concatenate per-core [5, 1024, 256] bf16 outputs along X, upcast.
"""
import numpy as np
import ml_dtypes

import concourse.bass as bass
import concourse.bacc as bacc
import concourse.mybir as mybir
from concourse import tile
from concourse.bass_utils import run_bass_kernel_spmd

BF16 = ml_dtypes.bfloat16
F32 = np.float32

Y, X = 1024, 2048
NCORES = 8
XS = X // NCORES            # 256
RAD = 12                    # gauss radius: int(4.0*3.0+0.5)
H = 2 * RAD + 1             # 25
W_IN = XS + 2 * H           # 306
W_ALG = XS + 2 * RAD        # 280
OFF1B = H - RAD             # 13: slab-coord offset of alg window
AP_CUT = 15
YT = Y // 128               # 8 y tiles
XT_IN = [(0, 128), (128, 128), (256, W_IN - 256)]       # x tiles of slab (128,128,50)
XT_ALG = [(0, 128), (128, 128), (256, W_ALG - 256)]     # x tiles of alg width (128,128,24)
HALF = 512                  # y' half width for stage a psum
WW = YT * W_ALG             # 2240: wide-op width
CH_ORDER = [4, 5, 6, 0, 3, 1, 2]   # s, v0, v1, m00, m11, m01, m10


def _gauss():
    r = RAD
    x = np.arange(-r, r + 1, dtype=np.float64)
    k = np.exp(-0.5 * (x / 3.0) ** 2)
    k = (k / k.sum()).astype(np.float64)
    dk = np.convolve(k, [-0.5, 0.0, 0.5])
    return k.astype(F32), dk.astype(F32)


KERN, DKERN = _gauss()      # 25 taps (rad 12), 27 taps (rad 13)


# ---------------- band submatrix machinery (host) ----------------
class BandPack:
    """Dedup + pack all band submatrices into one [128, K] bf16 constant."""

    def __init__(self):
        self.blocks = {}
        self.cols = []
        self.total = 0

    def add(self, sub):
        sub16 = np.ascontiguousarray(sub.astype(BF16))
        key = (sub16.shape, sub16.tobytes())
        if key not in self.blocks:
            pad = np.zeros((128, sub16.shape[1]), dtype=BF16)
            pad[: sub16.shape[0]] = sub16
            self.blocks[key] = self.total
            self.cols.append(pad)
            self.total += sub16.shape[1]
        return self.blocks[key]

    def packed(self):
        return np.concatenate(self.cols, axis=1)


def band_subs_y(pack, ker, scale):
    """Circular conv along y (1024). Returns per half h: list of
    (chunk k, col a, col b, packed offset, rows) for rhs = B[krows, h*512+a : h*512+b]."""
    r = ker.shape[0] // 2
    B = np.zeros((Y, Y), dtype=F32)
    for j in range(Y):
        for t in range(-r, r + 1):
            B[(j + t) % Y, j] = ker[r + t] * scale
    out = []
    for h in range(2):
        subs = []
        for k in range(YT):
            sub = B[k * 128:(k + 1) * 128, h * HALF:(h + 1) * HALF]
            cols = np.flatnonzero(np.any(sub != 0.0, axis=0))
            if cols.size == 0:
                continue
            a, b = int(cols[0]), int(cols[-1] + 1)
            assert b - a == cols.size
            off = pack.add(sub[:, a:b])
            subs.append((k, a, b, off, 128))
        out.append(subs)
    return out


def band_subs_x(pack, ker, scale, n_in, n_out, off_in):
    """conv along x: out[j] = sum_t kc[t] in[j + off_in + t].
    Returns list of (chunk k, col a, col b, packed offset, rows)."""
    r = ker.shape[0] // 2
    B = np.zeros((n_in, n_out), dtype=F32)
    for j in range(n_out):
        for t in range(-r, r + 1):
            i = j + off_in + t
            if 0 <= i < n_in:
                B[i, j] = ker[r + t] * scale
    subs = []
    nchunks = (n_in + 127) // 128
    for k in range(nchunks):
        rows = min(128, n_in - k * 128)
        sub = B[k * 128:k * 128 + rows, :]
        cols = np.flatnonzero(np.any(sub != 0.0, axis=0))
        if cols.size == 0:
            continue
        a, b = int(cols[0]), int(cols[-1] + 1)
        assert b - a == cols.size
        off = pack.add(sub[:, a:b])
        subs.append((k, a, b, off, rows))
    return subs


# channel order in slab: m00 m01 m10 m11 s v0 v1
# stage-1a D-variant scale per channel (folds signs), stage-1b DK scale per channel
CH_DY_SCALE = [-1.0, -1.0, -1.0, -1.0, -1.0, 1.0, 0.5]   # conv_y(DK) scale
CH_DX_SCALE = [-1.0, -1.0, -1.0, -1.0, -1.0, -0.5, 1.0]  # conv_x(DK) scale


def build_graph():
    pack = BandPack()
    sub_ky = band_subs_y(pack, KERN, 1.0)                 # shared smooth-y (also stage 4a)
    sub_kx = band_subs_x(pack, KERN, 1.0, W_IN, W_ALG, OFF1B)    # smooth-x for dy fields
    # dy via raw 3-tap diff of gyk: fold 0.5*CH_DY_SCALE into the 1b Kx band
    sub_kxd = {}
    for sc in sorted(set(0.5 * s for s in CH_DY_SCALE)):
        sub_kxd[sc] = band_subs_x(pack, KERN, sc, W_IN, W_ALG, OFF1B)
    sub_dkx = {}
    for sc in sorted(set(CH_DX_SCALE)):
        sub_dkx[sc] = band_subs_x(pack, DKERN, sc, W_IN, W_ALG, OFF1B)
    sub_kx4 = band_subs_x(pack, KERN, 1.0, W_ALG, XS, RAD)       # stage 4b
    bands_np = pack.packed()
    KTOT = bands_np.shape[1]

    nc = bacc.Bacc()
    x_ext = nc.declare_dram_parameter("x", [7, Y, W_IN], mybir.dt.bfloat16, isOutput=False)
    bands_ext = nc.declare_dram_parameter("bands", [128, KTOT], mybir.dt.bfloat16, isOutput=False)
    mask_ext = nc.declare_dram_parameter("mask", [128, 3], mybir.dt.float32, isOutput=False)
    out_ext = nc.declare_dram_parameter("out", [5, Y, XS], mybir.dt.bfloat16, isOutput=True)

    bf = mybir.dt.bfloat16
    f32 = mybir.dt.float32
    TT = mybir.AluOpType
    ACT_COPY = mybir.ActivationFunctionType.Copy

    with tile.TileContext(nc) as tc:
        with (
            tc.tile_pool(name="const", bufs=1) as constp,
            tc.tile_pool(name="slab", bufs=1) as slabp,
            tc.tile_pool(name="wide", bufs=1) as widep,
            tc.tile_pool(name="gyt", bufs=1) as gytp,
            tc.tile_pool(name="alg", bufs=2) as algp,
            tc.tile_pool(name="rb", bufs=2) as rbp,
            tc.tile_pool(name="pre", bufs=1) as prep,
            tc.tile_pool(name="gyt2", bufs=2) as gyt2p,
            tc.tile_pool(name="outs", bufs=1) as outsp,
            tc.tile_pool(name="psa", bufs=2, space=bass.MemorySpace.PSUM) as psap,
            tc.tile_pool(name="psg", bufs=2, space=bass.MemorySpace.PSUM) as psgp,
        ):
            bands = constp.tile([128, KTOT], bf, tag="bands", name="bands")
            nc.sync.dma_start(bands[:, :], bands_ext[:, :])
            maskt = constp.tile([128, 3], f32, tag="mask", name="mask")
            nc.sync.dma_start(maskt[:, :], mask_ext[:, :])

            # persistent slab: one wide tile per channel [128, YT*W_IN];
            # DMA in CH_ORDER so s/v land first.
            slabw = [slabp.tile([128, YT * W_IN], bf, tag=f"slabw{c}", name=f"slabw{c}")
                     for c in range(7)]
            for c in CH_ORDER:
                nc.sync.dma_start(
                    slabw[c][:, :],
                    x_ext[c].rearrange("(t p) x -> p t x", p=128))
            slab = [[slabw[c][:, t * W_IN:(t + 1) * W_IN] for t in range(YT)]
                    for c in range(7)]

            def sview(c):
                """[128, YT, W_ALG] strided view of channel c's alg window."""
                return slabw[c].rearrange("p (t x) -> p t x", x=W_IN)[:, :, OFF1B:OFF1B + W_ALG]

            def wtile(tag):
                return widep.tile([128, WW], bf, tag=tag, name=tag)

            def w3(t_):
                return t_.rearrange("p (t x) -> p t x", x=W_ALG)

            # ---- wide precompute: slab-only dependencies, overlaps input DMA ----
            # coefficient fields (affine in raw s) on ACT
            c1w = wtile("c1w"); c2w = wtile("c2w"); c3w = wtile("c3w"); c4w = wtile("c4w")
            nc.scalar.activation(w3(c1w), sview(4), ACT_COPY, bias=-0.11, scale=0.099)
            nc.scalar.activation(w3(c2w), sview(4), ACT_COPY, bias=0.767, scale=0.055)
            nc.scalar.activation(w3(c3w), sview(4), ACT_COPY, bias=0.732, scale=-0.59)
            nc.scalar.activation(w3(c4w), sview(4), ACT_COPY, bias=0.069, scale=-0.048)
            # trm / u1 / u2 on Pool (idle during DMA/1a)
            trmw = wtile("trmw"); u1w = wtile("u1w"); u2w = wtile("u2w")
            nc.gpsimd.scalar_tensor_tensor(w3(trmw), sview(0), 1.0, sview(3), TT.mult, TT.add)
            nc.gpsimd.scalar_tensor_tensor(w3(u1w), sview(1), 1.0, sview(2), TT.mult, TT.add)
            nc.gpsimd.scalar_tensor_tensor(w3(u2w), sview(3), 1.0, sview(0), TT.mult, TT.subtract)
            # ew = c1 + c3*trm (reuses c3w, c1w);  Ccw = c4*trm (reuses c4w)
            nc.vector.tensor_tensor(c3w[:, :], c3w[:, :], trmw[:, :], TT.mult)
            nc.vector.tensor_tensor(c1w[:, :], c1w[:, :], c3w[:, :], TT.add)
            nc.vector.tensor_tensor(c4w[:, :], c4w[:, :], trmw[:, :], TT.mult)
            ew, Ccw = c1w, c4w

            def conv_group(psum_ap, subs, lhsT_fn):
                n = len(subs)
                for i, (k, a, b, off, rows) in enumerate(subs):
                    nc.tensor.matmul(
                        psum_ap[:, a:b],
                        lhsT_fn(k, rows),
                        bands[:rows, off:off + b - a],
                        start=(i == 0),
                        stop=(i == n - 1),
                    )

            # ---------------- stage 1a machinery ----------------
            gyk_all = [None] * 7
            gyd_all = [None] * 7
            n1a = [0]

            def stage1a(c):
                gyk = [gytp.tile([128, 1024], bf, tag=f"gyk{c}_{xt}", name=f"gyk{c}_{xt}") for xt in range(3)]
                gyd = [gytp.tile([128, 1024], bf, tag=f"gyd{c}_{xt}", name=f"gyd{c}_{xt}") for xt in range(3)]
                gyk_all[c] = gyk
                gyd_all[c] = gyd
                for xt, (x0, xw) in enumerate(XT_IN):
                    ps = psap.tile([128, 2 * HALF], f32, tag="psa", name="psa")
                    for h in range(2):
                        conv_group(
                            ps[:xw, h * HALF:(h + 1) * HALF], sub_ky[h],
                            lambda k, rows: slab[c][k][:, x0:x0 + xw])
                    nc.scalar.copy(gyk[xt][:xw, :], ps[:xw, :])
                    g, d = gyk[xt], gyd[xt]
                    # interior diff: DVE for the s/v channels (they gate the
                    # w/s prologues), m channels split DVE/Pool
                    if c < 4 and n1a[0] % 2 == 0:
                        nc.gpsimd.scalar_tensor_tensor(
                            d[:xw, 1:1023], g[:xw, 2:1024], 1.0, g[:xw, 0:1022],
                            TT.mult, TT.subtract)
                    else:
                        nc.vector.tensor_tensor(d[:xw, 1:1023], g[:xw, 2:1024], g[:xw, 0:1022], TT.subtract)
                    # merged circular boundary fixup (cols 0 and 1023) on DVE
                    nc.vector.tensor_tensor(
                        d[:xw, 0:1024:1023], g[:xw, 1::-1], g[:xw, 1023:1021:-1],
                        TT.subtract)
                    n1a[0] += 1

            # ---------------- stage 1b machinery ----------------
            A = slice(OFF1B, OFF1B + W_ALG)
            dxsubs = {c: sub_dkx[CH_DX_SCALE[c]] for c in range(7)}

            def _emit(ps_view, t, specs):
                first = True
                for gi, (gt, subs) in enumerate(specs):
                    n = len(subs)
                    for i, (k, a, b, off, rows) in enumerate(subs):
                        nc.tensor.matmul(
                            ps_view[:, a:b],
                            gt[k][:rows, t * 128:(t + 1) * 128],
                            bands[:rows, off:off + b - a],
                            start=first,
                            stop=(gi == len(specs) - 1 and i == n - 1),
                        )
                        first = False

            def grad_pair(t, specsA, specsB):
                ps = psgp.tile([128, 2 * HALF], f32, tag="pg", name="pg")
                _emit(ps[:, 0:HALF], t, specsA)
                _emit(ps[:, HALF:2 * HALF], t, specsB)
                return ps

            # ---- 1a for s, v0, v1 ----
            for c in (4, 5, 6):
                stage1a(c)

            # ---- w/trE prologue: pairs for all 8 tiles -> wide tile ----
            # wide w and trE tiles; w reuses the dead trmw buffer (tag reuse)
            wfw = wtile("trmw")
            trEw = wtile("trEw")
            for t in range(YT):
                ps_wt = grad_pair(
                    t,
                    [(gyk_all[5], dxsubs[5]), (gyd_all[6], sub_kxd[0.5 * CH_DY_SCALE[6]])],
                    [(gyd_all[5], sub_kxd[0.5 * CH_DY_SCALE[5]]), (gyk_all[6], dxsubs[6])])
                nc.scalar.copy(wfw[:, t * W_ALG:(t + 1) * W_ALG], ps_wt[:, 0:W_ALG])
                nc.scalar.copy(trEw[:, t * W_ALG:(t + 1) * W_ALG], ps_wt[:, HALF:HALF + W_ALG])

            wview = w3(wfw)
            trEview = w3(trEw)
            # wu1 = w*u1 (in place into u1w), wu2 = w*u2 (into u2w)
            nc.vector.tensor_tensor(w3(u1w), w3(u1w), wview, TT.mult)
            nc.vector.tensor_tensor(w3(u2w), w3(u2w), wview, TT.mult)
            wu1w, wu2w = u1w, u2w
            # Ac = c2*trE + ew  (in place into c2w)
            nc.vector.tensor_tensor(w3(c2w), w3(c2w), trEview, TT.mult)
            nc.vector.tensor_tensor(c2w[:, :], c2w[:, :], ew[:, :], TT.add)
            Acw = c2w

            # ---- s prologue: sdot straight from PSUM into wide pre4 ----
            prew = [prep.tile([128, WW], bf, tag=f"prew{f}", name=f"prew{f}")
                    for f in range(5)]
            for t in range(YT):
                ps_s = grad_pair(t, [(gyd_all[4], sub_kxd[0.5 * CH_DY_SCALE[4]])],
                                 [(gyk_all[4], dxsubs[4])])
                sd1 = algp.tile([128, W_ALG], bf, tag="q1", name="sd1")
                nc.vector.tensor_tensor(sd1[:, :], ps_s[:, 0:W_ALG], slab[5][t][:, A], TT.mult)
                sd2 = algp.tile([128, W_ALG], bf, tag="q2", name="sd2")
                nc.gpsimd.scalar_tensor_tensor(
                    sd2[:, :], ps_s[:, HALF:HALF + W_ALG], 1.0, slab[6][t][:, A],
                    TT.mult, TT.mult)
                nc.vector.tensor_tensor(
                    prew[4][:, t * W_ALG:(t + 1) * W_ALG], sd1[:, :], sd2[:, :], TT.add)

            # ---- 1a for the m channels ----
            for c in (0, 3, 1, 2):
                stage1a(c)

            # ---- m-loop, channel-major ----
            for ch in range(4):
                # wide b = Ac*m +- w*u (+ Cc for ch0), off the critical chain
                bw = rbp.tile([128, WW], bf, tag="bw", name=f"bw{ch}")
                nc.vector.tensor_tensor(w3(bw), w3(Acw), sview(ch), TT.mult)
                if ch == 0:
                    nc.gpsimd.scalar_tensor_tensor(
                        bw[:, :], bw[:, :], 1.0, wu1w[:, :], TT.mult, TT.subtract)
                    nc.gpsimd.scalar_tensor_tensor(
                        bw[:, :], bw[:, :], 1.0, Ccw[:, :], TT.mult, TT.add)
                elif ch == 3:
                    nc.gpsimd.scalar_tensor_tensor(
                        bw[:, :], bw[:, :], 1.0, wu1w[:, :], TT.mult, TT.add)
                else:
                    nc.gpsimd.scalar_tensor_tensor(
                        bw[:, :], bw[:, :], 1.0, wu2w[:, :], TT.mult, TT.subtract)
                for t in range(YT):
                    ps_ab = grad_pair(t, [(gyd_all[ch], sub_kxd[0.5 * CH_DY_SCALE[ch]])],
                                      [(gyk_all[ch], dxsubs[ch])])
                    q1 = algp.tile([128, W_ALG], bf, tag="q1", name="q1")
                    q2 = algp.tile([128, W_ALG], bf, tag="q2", name="q2")
                    nc.vector.tensor_tensor(q1[:, :], ps_ab[:, 0:W_ALG], slab[5][t][:, A], TT.mult)
                    nc.gpsimd.scalar_tensor_tensor(
                        q2[:, :], ps_ab[:, HALF:HALF + W_ALG], 1.0, slab[6][t][:, A],
                        TT.mult, TT.mult)
                    nc.vector.tensor_tensor(q1[:, :], q1[:, :], q2[:, :], TT.add)
                    nc.vector.tensor_tensor(
                        prew[ch][:, t * W_ALG:(t + 1) * W_ALG], q1[:, :],
                        bw[:, t * W_ALG:(t + 1) * W_ALG], TT.add)

            # ---------------- stage 4: final smooth of 5 fields ----------------
            n4a = 0
            n4b = 0
            for f in range(5):
                gy2 = [[gyt2p.tile([128, HALF], bf, tag=f"gy2{xt}_{h}", name=f"gy2{xt}_{h}")
                        for h in range(2)] for xt in range(3)]
                for xt, (x0, xw) in enumerate(XT_ALG):
                    ps = psap.tile([128, 2 * HALF], f32, tag="psa", name="psa")
                    sc = maskt[:xw, xt:xt + 1]
                    for h in range(2):
                        conv_group(
                            ps[:xw, h * HALF:(h + 1) * HALF], sub_ky[h],
                            lambda k, rows: prew[f][:, k * W_ALG + x0:k * W_ALG + x0 + xw])
                        dst = gy2[xt][h][:xw, :]
                        srcv = ps[:xw, h * HALF:(h + 1) * HALF]
                        if n4a % 3 == 0:
                            nc.scalar.activation(dst, srcv, ACT_COPY, scale=sc)
                        elif n4a % 3 == 1:
                            nc.vector.tensor_scalar(dst, srcv, sc, None, TT.mult)
                        else:
                            nc.gpsimd.tensor_scalar(dst, srcv, sc, None, TT.mult)
                        n4a += 1
                for hh in range(2):
                    ow = outsp.tile([128, 4 * XS], bf, tag=f"ow{hh}", name=f"ow{hh}")
                    for tp in range(2):
                        ps = psgp.tile([128, 2 * HALF], f32, tag="pg", name="pg")
                        for half in range(2):
                            t = 4 * hh + 2 * tp + half
                            conv_group(
                                ps[:, half * HALF:half * HALF + XS], sub_kx4,
                                lambda k, rows: gy2[k][t // 4][:rows, (t % 4) * 128:(t % 4 + 1) * 128])
                        dst = ow.rearrange("p (t x) -> p t x", x=XS)[:, 2 * tp:2 * tp + 2, :]
                        src = ps.rearrange("p (t x) -> p t x", x=HALF)[:, :, 0:XS]
                        if n4b % 4 == 0:
                            nc.vector.tensor_copy(dst, src)
                        elif n4b % 4 == 2:
                            nc.gpsimd.tensor_copy(dst, src)
                        else:
                            nc.scalar.copy(dst, src)
                        n4b += 1
                    nc.sync.dma_start(
                        out_ext[f, hh * HALF:(hh + 1) * HALF, :].rearrange(
                            "(t p) x -> p t x", p=128),
                        ow[:, :])

    nc.compile()
    return nc, bands_np


_CACHE = {}


def _get_graph():
    if "nc" not in _CACHE:
        _CACHE["nc"], _CACHE["bands"] = build_graph()
    return _CACHE["nc"], _CACHE["bands"]


def host_prep(y, v):
    m = y[:4]
    s = y[4:5]
    v_lr = v[:, ::-1, :].copy()
    v_lr[0] *= -1.0
    vs = 0.5 * (v + v_lr)
    f = np.concatenate([m, s, vs], axis=0).astype(F32)      # [7, Y, X]
    fp = np.pad(f, ((0, 0), (0, 0), (H, H)), mode='edge')
    slabs, masks = [], []
    for c in range(NCORES):
        x0 = c * XS
        slabs.append(np.ascontiguousarray(fp[:, :, x0:x0 + W_IN]).astype(BF16))
        g = x0 + np.arange(W_ALG) - RAD
        mk = ((g >= AP_CUT) & (g < X - AP_CUT)).astype(F32)
        mk_t = np.zeros((128, 3), dtype=F32)
        for xt, (a, w) in enumerate(XT_ALG):
            mk_t[:w, xt] = mk[a:a + w]
        masks.append(mk_t)
    return slabs, masks


def kernel(y, v):
    y = np.asarray(y, dtype=F32)
    v = np.asarray(v, dtype=F32)
    nc, bands_np = _get_graph()
    slabs, masks = host_prep(y, v)
    in_maps = [
        {"x": slabs[c], "bands": bands_np, "mask": masks[c]}
        for c in range(NCORES)
    ]
    res = run_bass_kernel_spmd(nc, in_maps, core_ids=list(range(NCORES)))
    out = np.concatenate([res.results[c]["out"] for c in range(NCORES)], axis=2)
    return out.astype(F32)


# revision 17
# speedup vs baseline: 1.1162x; 1.0004x over previous
"""Distributed Trainium2 Bass kernel for nn_ClosedFlyLoop.

Strategy (8 NeuronCores, shard X into 8 blocks of 256):
 - host: symmetrize v, split y into (m[4], s), pad X edge-replicate by H=25,
   cut per-core overlapping slabs [7, 1024, 306] (halo covers smooth+diff+
   post-smooth supports; the AP_CUT zeroing erases the only global-edge
   discrepancy), convert to bf16.
 - device, per core (no collectives needed):
     input DMA ordered (s, v0, v1, m...) so the coefficient/velocity wide
       precomputes and the s/v conv chains start inside the DMA window.
     stage 1a: conv along y (circular) as banded matmuls that also transpose
       layout [y,x] -> [x,y']; 2-bank [128,1024] psum groups with single ACT
       evacs; dy via circular 3-tap diff (DVE, overflow to Pool).
     w/trE prologue: the vorticity/strain pairs for all 8 y-tiles run right
       after the v channels' 1a, evac into one wide tile; wu1/wu2/t1/Ac are
       then single [128, 8*280] wide ops (buffers reused in place).
     s prologue: sdot products straight from PSUM into a wide pre field.
     m-loop (channel-major): per channel a wide b = Ac*m +- w*u field on
       Pool, then per y-tile the PSUM gradient pair is consumed directly
       (q1 DVE / q2 Pool-STT) and folded with two adds.
     stage 4a/4b: final smooth; 4a on psa pool, 4b on psg pool, per-half gy2
       tiles, mask folded into 4a evacs, outputs packed and DMAd per y-half.
 - host: concatenate per-core [5, 1024, 256] bf16 outputs along X, upcast.
"""
import numpy as np
import ml_dtypes

import concourse.bass as bass
import concourse.bacc as bacc
import concourse.mybir as mybir
from concourse import tile
from concourse.bass_utils import run_bass_kernel_spmd

BF16 = ml_dtypes.bfloat16
F32 = np.float32

Y, X = 1024, 2048
NCORES = 8
XS = X // NCORES            # 256
RAD = 12                    # gauss radius: int(4.0*3.0+0.5)
H = 2 * RAD + 1             # 25
W_IN = XS + 2 * H           # 306
W_ALG = XS + 2 * RAD        # 280
OFF1B = H - RAD             # 13: slab-coord offset of alg window
AP_CUT = 15
YT = Y // 128               # 8 y tiles
XT_IN = [(0, 128), (128, 128), (256, W_IN - 256)]       # x tiles of slab (128,128,50)
XT_ALG = [(0, 128), (128, 128), (256, W_ALG - 256)]     # x tiles of alg width (128,128,24)
HALF = 512                  # y' half width for stage a psum
WW = YT * W_ALG             # 2240: wide-op width
CH_ORDER = [4, 5, 6, 0, 3, 1, 2]   # s, v0, v1, m00, m11, m01, m10


def _gauss():
    r = RAD
    x = np.arange(-r, r + 1, dtype=np.float64)
    k = np.exp(-0.5 * (x / 3.0) ** 2)
    k = (k / k.sum()).astype(np.float64)
    dk = np.convolve(k, [-0.5, 0.0, 0.5])
    return k.astype(F32), dk.astype(F32)


KERN, DKERN = _gauss()      # 25 taps (rad 12), 27 taps (rad 13)


# ---------------- band submatrix machinery (host) ----------------
class BandPack:
    """Dedup + pack all band submatrices into one [128, K] bf16 constant."""

    def __init__(self):
        self.blocks = {}
        self.cols = []
        self.total = 0

    def add(self, sub):
        sub16 = np.ascontiguousarray(sub.astype(BF16))
        key = (sub16.shape, sub16.tobytes())
        if key not in self.blocks:
            pad = np.zeros((128, sub16.shape[1]), dtype=BF16)
            pad[: sub16.shape[0]] = sub16
            self.blocks[key] = self.total
            self.cols.append(pad)
            self.total += sub16.shape[1]
        return self.blocks[key]

    def packed(self):
        return np.concatenate(self.cols, axis=1)


def band_subs_y(pack, ker, scale):
    """Circular conv along y (1024). Returns per half h: list of
    (chunk k, col a, col b, packed offset, rows) for rhs = B[krows, h*512+a : h*512+b]."""
    r = ker.shape[0] // 2
    B = np.zeros((Y, Y), dtype=F32)
    for j in range(Y):
        for t in range(-r, r + 1):
            B[(j + t) % Y, j] = ker[r + t] * scale
    out = []
    for h in range(2):
        subs = []
        for k in range(YT):
            sub = B[k * 128:(k + 1) * 128, h * HALF:(h + 1) * HALF]
            cols = np.flatnonzero(np.any(sub != 0.0, axis=0))
            if cols.size == 0:
                continue
            a, b = int(cols[0]), int(cols[-1] + 1)
            assert b - a == cols.size
            off = pack.add(sub[:, a:b])
            subs.append((k, a, b, off, 128))
        out.append(subs)
    return out


def band_subs_x(pack, ker, scale, n_in, n_out, off_in):
    """conv along x: out[j] = sum_t kc[t] in[j + off_in + t].
    Returns list of (chunk k, col a, col b, packed offset, rows)."""
    r = ker.shape[0] // 2
    B = np.zeros((n_in, n_out), dtype=F32)
    for j in range(n_out):
        for t in range(-r, r + 1):
            i = j + off_in + t
            if 0 <= i < n_in:
                B[i, j] = ker[r + t] * scale
    subs = []
    nchunks = (n_in + 127) // 128
    for k in range(nchunks):
        rows = min(128, n_in - k * 128)
        sub = B[k * 128:k * 128 + rows, :]
        cols = np.flatnonzero(np.any(sub != 0.0, axis=0))
        if cols.size == 0:
            continue
        a, b = int(cols[0]), int(cols[-1] + 1)
        assert b - a == cols.size
        off = pack.add(sub[:, a:b])
        subs.append((k, a, b, off, rows))
    return subs


# channel order in slab: m00 m01 m10 m11 s v0 v1
# stage-1a D-variant scale per channel (folds signs), stage-1b DK scale per channel
CH_DY_SCALE = [-1.0, -1.0, -1.0, -1.0, -1.0, 1.0, 0.5]   # conv_y(DK) scale
CH_DX_SCALE = [-1.0, -1.0, -1.0, -1.0, -1.0, -0.5, 1.0]  # conv_x(DK) scale


def build_graph():
    pack = BandPack()
    sub_ky = band_subs_y(pack, KERN, 1.0)                 # shared smooth-y (also stage 4a)
    sub_kx = band_subs_x(pack, KERN, 1.0, W_IN, W_ALG, OFF1B)    # smooth-x for dy fields
    # dy via raw 3-tap diff of gyk: fold 0.5*CH_DY_SCALE into the 1b Kx band
    sub_kxd = {}
    for sc in sorted(set(0.5 * s for s in CH_DY_SCALE)):
        sub_kxd[sc] = band_subs_x(pack, KERN, sc, W_IN, W_ALG, OFF1B)
    sub_dkx = {}
    for sc in sorted(set(CH_DX_SCALE)):
        sub_dkx[sc] = band_subs_x(pack, DKERN, sc, W_IN, W_ALG, OFF1B)
    sub_kx4 = band_subs_x(pack, KERN, 1.0, W_ALG, XS, RAD)       # stage 4b
    bands_np = pack.packed()
    KTOT = bands_np.shape[1]

    nc = bacc.Bacc()
    x_ext = nc.declare_dram_parameter("x", [7, Y, W_IN], mybir.dt.bfloat16, isOutput=False)
    bands_ext = nc.declare_dram_parameter("bands", [128, KTOT], mybir.dt.bfloat16, isOutput=False)
    mask_ext = nc.declare_dram_parameter("mask", [128, 3], mybir.dt.float32, isOutput=False)
    out_ext = nc.declare_dram_parameter("out", [5, Y, XS], mybir.dt.bfloat16, isOutput=True)

    bf = mybir.dt.bfloat16
    f32 = mybir.dt.float32
    TT = mybir.AluOpType
    ACT_COPY = mybir.ActivationFunctionType.Copy

    with tile.TileContext(nc) as tc:
        with (
            tc.tile_pool(name="const", bufs=1) as constp,
            tc.tile_pool(name="slab", bufs=1) as slabp,
            tc.tile_pool(name="wide", bufs=1) as widep,
            tc.tile_pool(name="gyt", bufs=1) as gytp,
            tc.tile_pool(name="alg", bufs=2) as algp,
            tc.tile_pool(name="rb", bufs=1) as rbp,
            tc.tile_pool(name="pre", bufs=1) as prep,
            tc.tile_pool(name="gyt2", bufs=2) as gyt2p,
            tc.tile_pool(name="outs", bufs=1) as outsp,
            tc.tile_pool(name="psa", bufs=2, space=bass.MemorySpace.PSUM) as psap,
            tc.tile_pool(name="psg", bufs=2, space=bass.MemorySpace.PSUM) as psgp,
        ):
            bands = constp.tile([128, KTOT], bf, tag="bands", name="bands")
            nc.sync.dma_start(bands[:, :], bands_ext[:, :])
            maskt = constp.tile([128, 3], f32, tag="mask", name="mask")
            nc.sync.dma_start(maskt[:, :], mask_ext[:, :])

            # persistent slab: one wide tile per channel [128, YT*W_IN];
            # DMA in CH_ORDER so s/v land first.
            slabw = [slabp.tile([128, YT * W_IN], bf, tag=f"slabw{c}", name=f"slabw{c}")
                     for c in range(7)]
            for c in CH_ORDER:
                nc.sync.dma_start(
                    slabw[c][:, :],
                    x_ext[c].rearrange("(t p) x -> p t x", p=128))
            slab = [[slabw[c][:, t * W_IN:(t + 1) * W_IN] for t in range(YT)]
                    for c in range(7)]

            def sview(c):
                """[128, YT, W_ALG] strided view of channel c's alg window."""
                return slabw[c].rearrange("p (t x) -> p t x", x=W_IN)[:, :, OFF1B:OFF1B + W_ALG]

            def wtile(tag):
                return widep.tile([128, WW], bf, tag=tag, name=tag)

            def w3(t_):
                return t_.rearrange("p (t x) -> p t x", x=W_ALG)

            # ---- wide precompute: slab-only dependencies, overlaps input DMA ----
            # coefficient fields (affine in raw s): split ACT / DVE
            c1w = wtile("c1w"); c2w = wtile("c2w"); c3w = wtile("c3w"); c4w = wtile("c4w")
            nc.vector.tensor_scalar(w3(c1w), sview(4), 0.099, -0.11, TT.mult, TT.add)
            nc.scalar.activation(w3(c2w), sview(4), ACT_COPY, bias=0.767, scale=0.055)
            nc.vector.tensor_scalar(w3(c3w), sview(4), -0.59, 0.732, TT.mult, TT.add)
            nc.scalar.activation(w3(c4w), sview(4), ACT_COPY, bias=0.069, scale=-0.048)
            # trm / u1 / u2 on Pool (idle during DMA/1a)
            trmw = wtile("trmw"); u1w = wtile("u1w"); u2w = wtile("u2w")
            nc.gpsimd.scalar_tensor_tensor(w3(trmw), sview(0), 1.0, sview(3), TT.mult, TT.add)
            nc.gpsimd.scalar_tensor_tensor(w3(u1w), sview(1), 1.0, sview(2), TT.mult, TT.add)
            nc.gpsimd.scalar_tensor_tensor(w3(u2w), sview(3), 1.0, sview(0), TT.mult, TT.subtract)
            # ew = c1 + c3*trm (reuses c3w, c1w);  Ccw = c4*trm (reuses c4w)
            nc.vector.tensor_tensor(c3w[:, :], c3w[:, :], trmw[:, :], TT.mult)
            nc.vector.tensor_tensor(c1w[:, :], c1w[:, :], c3w[:, :], TT.add)
            nc.vector.tensor_tensor(c4w[:, :], c4w[:, :], trmw[:, :], TT.mult)
            ew, Ccw = c1w, c4w

            def conv_group(psum_ap, subs, lhsT_fn):
                n = len(subs)
                for i, (k, a, b, off, rows) in enumerate(subs):
                    nc.tensor.matmul(
                        psum_ap[:, a:b],
                        lhsT_fn(k, rows),
                        bands[:rows, off:off + b - a],
                        start=(i == 0),
                        stop=(i == n - 1),
                    )

            # ---------------- stage 1a machinery ----------------
            gyk_all = [None] * 7
            gyd_all = [None] * 7
            n1a = [0]

            def stage1a(c):
                gyk = [gytp.tile([128, 1024], bf, tag=f"gyk{c}_{xt}", name=f"gyk{c}_{xt}") for xt in range(3)]
                gyd = [gytp.tile([128, 1024], bf, tag=f"gyd{c}_{xt}", name=f"gyd{c}_{xt}") for xt in range(3)]
                gyk_all[c] = gyk
                gyd_all[c] = gyd
                for xt, (x0, xw) in enumerate(XT_IN):
                    ps = psap.tile([128, 2 * HALF], f32, tag="psa", name="psa")
                    for h in range(2):
                        conv_group(
                            ps[:xw, h * HALF:(h + 1) * HALF], sub_ky[h],
                            lambda k, rows: slab[c][k][:, x0:x0 + xw])
                    g, d = gyk[xt], gyd[xt]
                    if c >= 4:
                        # s/v phase: ACT is busy with the coefficient wides,
                        # DVE idles -> evac on DVE, diffs on DVE
                        nc.vector.tensor_copy(g[:xw, :], ps[:xw, :])
                        nc.vector.tensor_tensor(d[:xw, 1:1023], g[:xw, 2:1024], g[:xw, 0:1022], TT.subtract)
                    else:
                        # m phase: evacs round-robin ACT/DVE/Pool, diffs split
                        e = n1a[0] % 3
                        if e == 0:
                            nc.scalar.copy(g[:xw, :], ps[:xw, :])
                        elif e == 1:
                            nc.vector.tensor_copy(g[:xw, :], ps[:xw, :])
                        else:
                            nc.gpsimd.tensor_copy(g[:xw, :], ps[:xw, :])
                        if n1a[0] % 2 == 0:
                            nc.gpsimd.scalar_tensor_tensor(
                                d[:xw, 1:1023], g[:xw, 2:1024], 1.0, g[:xw, 0:1022],
                                TT.mult, TT.subtract)
                        else:
                            nc.vector.tensor_tensor(d[:xw, 1:1023], g[:xw, 2:1024], g[:xw, 0:1022], TT.subtract)
                        n1a[0] += 1
                    # merged circular boundary fixup (cols 0 and 1023) on DVE
                    nc.vector.tensor_tensor(
                        d[:xw, 0:1024:1023], g[:xw, 1::-1], g[:xw, 1023:1021:-1],
                        TT.subtract)

            # ---------------- stage 1b machinery ----------------
            A = slice(OFF1B, OFF1B + W_ALG)
            dxsubs = {c: sub_dkx[CH_DX_SCALE[c]] for c in range(7)}

            def _emit(ps_view, t, specs):
                first = True
                for gi, (gt, subs) in enumerate(specs):
                    n = len(subs)
                    for i, (k, a, b, off, rows) in enumerate(subs):
                        nc.tensor.matmul(
                            ps_view[:, a:b],
                            gt[k][:rows, t * 128:(t + 1) * 128],
                            bands[:rows, off:off + b - a],
                            start=first,
                            stop=(gi == len(specs) - 1 and i == n - 1),
                        )
                        first = False

            def grad_pair(t, specsA, specsB):
                ps = psgp.tile([128, 2 * HALF], f32, tag="pg", name="pg")
                _emit(ps[:, 0:HALF], t, specsA)
                _emit(ps[:, HALF:2 * HALF], t, specsB)
                return ps

            # ---- 1a for s, v0, v1 ----
            for c in (4, 5, 6):
                stage1a(c)

            # ---- w/trE prologue: pairs for all 8 tiles -> wide tile ----
            # wide w and trE tiles; w reuses the dead trmw buffer (tag reuse)
            wfw = wtile("trmw")
            trEw = wtile("trEw")
            for t in range(YT):
                ps_wt = grad_pair(
                    t,
                    [(gyk_all[5], dxsubs[5]), (gyd_all[6], sub_kxd[0.5 * CH_DY_SCALE[6]])],
                    [(gyd_all[5], sub_kxd[0.5 * CH_DY_SCALE[5]]), (gyk_all[6], dxsubs[6])])
                nc.scalar.copy(wfw[:, t * W_ALG:(t + 1) * W_ALG], ps_wt[:, 0:W_ALG])
                nc.vector.tensor_copy(trEw[:, t * W_ALG:(t + 1) * W_ALG], ps_wt[:, HALF:HALF + W_ALG])

            wview = w3(wfw)
            trEview = w3(trEw)
            # wu1 = w*u1 (in place into u1w), wu2 = w*u2 (into u2w)
            nc.vector.tensor_tensor(w3(u1w), w3(u1w), wview, TT.mult)
            nc.vector.tensor_tensor(w3(u2w), w3(u2w), wview, TT.mult)
            wu1w, wu2w = u1w, u2w
            # Ac = c2*trE + ew  (in place into c2w)
            nc.vector.tensor_tensor(w3(c2w), w3(c2w), trEview, TT.mult)
            nc.vector.tensor_tensor(c2w[:, :], c2w[:, :], ew[:, :], TT.add)
            Acw = c2w

            # ---- s prologue: sdot straight from PSUM into wide pre4 ----
            prew = [prep.tile([128, WW], bf, tag=f"prew{f}", name=f"prew{f}")
                    for f in range(5)]
            for t in range(YT):
                ps_s = grad_pair(t, [(gyd_all[4], sub_kxd[0.5 * CH_DY_SCALE[4]])],
                                 [(gyk_all[4], dxsubs[4])])
                sd1 = algp.tile([128, W_ALG], bf, tag="q1", name="sd1")
                nc.vector.tensor_tensor(sd1[:, :], ps_s[:, 0:W_ALG], slab[5][t][:, A], TT.mult)
                sd2 = algp.tile([128, W_ALG], bf, tag="q2", name="sd2")
                nc.gpsimd.scalar_tensor_tensor(
                    sd2[:, :], ps_s[:, HALF:HALF + W_ALG], 1.0, slab[6][t][:, A],
                    TT.mult, TT.mult)
                nc.vector.tensor_tensor(
                    prew[4][:, t * W_ALG:(t + 1) * W_ALG], sd1[:, :], sd2[:, :], TT.add)

            # ---- 1a for the m channels ----
            for c in (0, 3, 1, 2):
                stage1a(c)

            # ---- m-loop, channel-major ----
            # ch 0/1: gradients consumed straight from PSUM (q1 DVE, q2 Pool);
            # ch 2/3: ACT evacuates the pair (ACT is otherwise idle here) and
            # the products run on DVE at SBUF 2x rates.
            for ch in range(4):
                # wide b = Ac*m +- w*u (+ Cc for ch0), off the critical chain
                bw = rbp.tile([128, WW], bf, tag="bw", name=f"bw{ch}")
                nc.vector.tensor_tensor(w3(bw), w3(Acw), sview(ch), TT.mult)
                if ch == 0:
                    nc.vector.tensor_tensor(bw[:, :], bw[:, :], wu1w[:, :], TT.subtract)
                    nc.gpsimd.scalar_tensor_tensor(
                        bw[:, :], bw[:, :], 1.0, Ccw[:, :], TT.mult, TT.add)
                elif ch == 3:
                    nc.gpsimd.scalar_tensor_tensor(
                        bw[:, :], bw[:, :], 1.0, wu1w[:, :], TT.mult, TT.add)
                else:
                    nc.vector.tensor_tensor(bw[:, :], bw[:, :], wu2w[:, :], TT.subtract)
                for t in range(YT):
                    ps_ab = grad_pair(t, [(gyd_all[ch], sub_kxd[0.5 * CH_DY_SCALE[ch]])],
                                      [(gyk_all[ch], dxsubs[ch])])
                    q1 = algp.tile([128, W_ALG], bf, tag="q1", name="q1")
                    q2 = algp.tile([128, W_ALG], bf, tag="q2", name="q2")
                    if ch >= 2:
                        gab = algp.tile([128, 2 * W_ALG], bf, tag="gab", name="gab")
                        srcv = ps_ab.rearrange("p (b x) -> p b x", b=2)[:, :, 0:W_ALG]
                        dstv = gab.rearrange("p (b x) -> p b x", x=W_ALG)
                        nc.scalar.copy(dstv, srcv)
                        nc.vector.tensor_tensor(q1[:, :], gab[:, 0:W_ALG], slab[5][t][:, A], TT.mult)
                        nc.vector.tensor_tensor(q2[:, :], gab[:, W_ALG:2 * W_ALG], slab[6][t][:, A], TT.mult)
                    else:
                        nc.vector.tensor_tensor(q1[:, :], ps_ab[:, 0:W_ALG], slab[5][t][:, A], TT.mult)
                        nc.gpsimd.scalar_tensor_tensor(
                            q2[:, :], ps_ab[:, HALF:HALF + W_ALG], 1.0, slab[6][t][:, A],
                            TT.mult, TT.mult)
                    nc.vector.tensor_tensor(q1[:, :], q1[:, :], q2[:, :], TT.add)
                    nc.vector.tensor_tensor(
                        prew[ch][:, t * W_ALG:(t + 1) * W_ALG], q1[:, :],
                        bw[:, t * W_ALG:(t + 1) * W_ALG], TT.add)

            # ---------------- stage 4: final smooth of 5 fields ----------------
            n4a = 0
            n4b = 0
            for f in range(5):
                gy2 = [[gyt2p.tile([128, HALF], bf, tag=f"gy2{xt}_{h}", name=f"gy2{xt}_{h}")
                        for h in range(2)] for xt in range(3)]
                for xt, (x0, xw) in enumerate(XT_ALG):
                    ps = psap.tile([128, 2 * HALF], f32, tag="psa", name="psa")
                    sc = maskt[:xw, xt:xt + 1]
                    for h in range(2):
                        conv_group(
                            ps[:xw, h * HALF:(h + 1) * HALF], sub_ky[h],
                            lambda k, rows: prew[f][:, k * W_ALG + x0:k * W_ALG + x0 + xw])
                        dst = gy2[xt][h][:xw, :]
                        srcv = ps[:xw, h * HALF:(h + 1) * HALF]
                        if n4a % 3 == 0:
                            nc.scalar.activation(dst, srcv, ACT_COPY, scale=sc)
                        elif n4a % 3 == 1:
                            nc.vector.tensor_scalar(dst, srcv, sc, None, TT.mult)
                        else:
                            nc.gpsimd.tensor_scalar(dst, srcv, sc, None, TT.mult)
                        n4a += 1
                for hh in range(2):
                    ow = outsp.tile([128, 4 * XS], bf, tag=f"ow{hh}", name=f"ow{hh}")
                    for tp in range(2):
                        ps = psgp.tile([128, 2 * HALF], f32, tag="pg", name="pg")
                        for half in range(2):
                            t = 4 * hh + 2 * tp + half
                            conv_group(
                                ps[:, half * HALF:half * HALF + XS], sub_kx4,
                                lambda k, rows: gy2[k][t // 4][:rows, (t % 4) * 128:(t % 4 + 1) * 128])
                        dst = ow.rearrange("p (t x) -> p t x", x=XS)[:, 2 * tp:2 * tp + 2, :]
                        src = ps.rearrange("p (t x) -> p t x", x=HALF)[:, :, 0:XS]
                        if n4b % 4 == 0:
                            nc.vector.tensor_copy(dst, src)
                        elif n4b % 4 == 2:
                            nc.gpsimd.tensor_copy(dst, src)
                        else:
                            nc.scalar.copy(dst, src)
                        n4b += 1
                    nc.sync.dma_start(
                        out_ext[f, hh * HALF:(hh + 1) * HALF, :].rearrange(
                            "(t p) x -> p t x", p=128),
                        ow[:, :])

    nc.compile()
    return nc, bands_np


_CACHE = {}


def _get_graph():
    if "nc" not in _CACHE:
        _CACHE["nc"], _CACHE["bands"] = build_graph()
    return _CACHE["nc"], _CACHE["bands"]


def host_prep(y, v):
    m = y[:4]
    s = y[4:5]
    v_lr = v[:, ::-1, :].copy()
    v_lr[0] *= -1.0
    vs = 0.5 * (v + v_lr)
    f = np.concatenate([m, s, vs], axis=0).astype(F32)      # [7, Y, X]
    fp = np.pad(f, ((0, 0), (0, 0), (H, H)), mode='edge')
    slabs, masks = [], []
    for c in range(NCORES):
        x0 = c * XS
        slabs.append(np.ascontiguousarray(fp[:, :, x0:x0 + W_IN]).astype(BF16))
        g = x0 + np.arange(W_ALG) - RAD
        mk = ((g >= AP_CUT) & (g < X - AP_CUT)).astype(F32)
        mk_t = np.zeros((128, 3), dtype=F32)
        for xt, (a, w) in enumerate(XT_ALG):
            mk_t[:w, xt] = mk[a:a + w]
        masks.append(mk_t)
    return slabs, masks


def kernel(y, v):
    y = np.asarray(y, dtype=F32)
    v = np.asarray(v, dtype=F32)
    nc, bands_np = _get_graph()
    slabs, masks = host_prep(y, v)
    in_maps = [
        {"x": slabs[c], "bands": bands_np, "mask": masks[c]}
        for c in range(NCORES)
    ]
    res = run_bass_kernel_spmd(nc, in_maps, core_ids=list(range(NCORES)))
    out = np.concatenate([res.results[c]["out"] for c in range(NCORES)], axis=2)
    return out.astype(F32)
